# revision 2
# baseline (speedup 1.0000x reference)
"""Trainium2 Bass kernel for nn_DistinctionLoss (GFTT corners BCE + relu-cosine mean).

v2: batch-sharded 2 images/core across 8 cores.
 - fp8 DoubleRow raw gram (host-side e4m3 cast + d-major slab layout);
   normalization folded in post-relu via N=1 PE matvecs with r = rsqrt(diag).
 - GFTT restructured: (dx^2+dy^2, dx^2-dy^2) pushed through the linear gaussian
   convs, deleting the tr/A add/sub stages.
 - Elementwise spread across Act/DVE/Pool; bf16 everywhere DVE gets 2x mode.
"""
import os
import numpy as np
import ml_dtypes

import concourse.bacc as bacc
import concourse.mybir as mybir
from concourse.tile import TileContext
from concourse.bass_utils import run_bass_kernel_spmd

F32 = mybir.dt.float32
BF16 = mybir.dt.bfloat16
FP8 = mybir.dt.float8e4
AF = mybir.ActivationFunctionType
ALU = mybir.AluOpType
AX = mybir.AxisListType
DR = mybir.MatmulPerfMode.DoubleRow

H = W = 384
NIMG = 2
NDESC = 2048
DDIM = 256
NUM = 200
NEG = -1e30
BW = 136  # packed band window width

_bf = lambda a: np.ascontiguousarray(a.astype(ml_dtypes.bfloat16))


def _band(k, mode, n=384):
    pad = len(k) // 2
    idx = np.arange(n + 2 * pad) - pad
    if mode == "edge":
        src = np.clip(idx, 0, n - 1)
    else:  # reflect
        src = np.abs(idx)
        src = np.where(src >= n, 2 * (n - 1) - src, src)
    M = np.zeros((n, n), np.float32)
    for i, kv in enumerate(k):
        M[src[np.arange(n) + i], np.arange(n)] += kv
    return M


def _gauss7():
    xs = np.arange(7, dtype=np.float32) - 3.0
    g = np.exp(-0.5 * xs ** 2)
    return (g / g.sum()).astype(np.float32)


def _wins(M, nchunk):
    wins = []
    for k in range(nchunk):
        rows = M[k * 128:(k + 1) * 128]
        nz = np.nonzero(np.any(rows != 0, axis=0))[0]
        wins.append((int(nz[0]), int(nz[-1]) + 1) if len(nz) else None)
    return wins


def _pack(M, nchunk, wins):
    """Pack band matrix rows into [nchunk*128, BW] windows."""
    P = np.zeros((nchunk * 128, BW), np.float32)
    for k in range(nchunk):
        if wins[k] is None:
            continue
        c0, c1 = wins[k]
        P[k * 128:(k + 1) * 128, 0:c1 - c0] = M[k * 128:(k + 1) * 128, c0:c1]
    return P


def _nzpairs(M):
    out = []
    for ob in range(3):
        for kc in range(3):
            if np.any(M[kc * 128:(kc + 1) * 128, ob * 128:(ob + 1) * 128]):
                out.append((kc, ob))
    return out


def _consts():
    c = {}
    Msm = _band(np.array([1, 2, 1], np.float32) / 8.0, "edge")
    Mdf = _band(np.array([-1, 0, 1], np.float32), "edge")
    Mga = _band(_gauss7(), "reflect")
    coef = np.array([0.299, 0.587, 0.114], np.float32)
    b1s = np.concatenate([coef[i] * Msm for i in range(3)], axis=0)
    b1d = np.concatenate([coef[i] * Mdf for i in range(3)], axis=0)
    c["w1s"] = _wins(b1s, 9)
    c["w1d"] = _wins(b1d, 9)
    c["wga3"] = _wins(Mga, 3)
    c["pr_df"] = _nzpairs(Mdf)
    c["pr_sm"] = _nzpairs(Msm)
    c["pr_ga"] = _nzpairs(Mga)
    c["b1sp"] = _bf(_pack(b1s, 9, c["w1s"]))
    c["b1dp"] = _bf(_pack(b1d, 9, c["w1d"]))
    c["mgap"] = _bf(_pack(Mga, 3, c["wga3"]))
    c["msm"] = _bf(Msm)
    c["mdf"] = _bf(Mdf)
    c["mga"] = _bf(Mga)
    S8 = np.zeros((128, 16), np.float32)
    S8[np.arange(16) * 8, np.arange(16)] = 1.0
    c["s8"] = _bf(S8)
    T16 = np.zeros((16, 128), np.float32)
    T16[np.arange(128) // 8, np.arange(128)] = 1.0
    c["t16"] = _bf(T16)
    c["id2"] = np.eye(2, dtype=np.float32)
    c["ninfh"] = _bf(np.full((128, 384), NEG, np.float32))
    c["lw0"] = np.array([[0.0, 0.25 / 64.0], [0.0, 0.25 / 64.0]], np.float32)
    halves = np.zeros((2, 128), np.float32)
    halves[0, :64] = 1.0
    halves[1, 64:] = 1.0
    c["e2b"] = halves.copy()
    c["e64"] = np.ascontiguousarray(halves.T)
    c["iota128"] = (np.arange(128, dtype=np.float32) % 64).reshape(128, 1)
    c["ones128"] = np.ones((128, 1), np.float32)
    c["ones64h"] = _bf(np.ones((1, 64), np.float32))
    c["ones1"] = np.ones((1, 128), np.float32)
    return c


def build_program():
    C = _consts()
    nc = bacc.Bacc()

    imgs_d = nc.dram_tensor("imgs", [NIMG, 3, H, W], BF16, kind="ExternalInput")
    sd_d = nc.dram_tensor("sd", [NIMG, H, W], BF16, kind="ExternalInput")
    dsl_d = nc.dram_tensor("dsl", [NIMG, 128, 2, NDESC], FP8, kind="ExternalInput")
    dnm_d = nc.dram_tensor("dnm", [NIMG, 128, 16, DDIM], FP8, kind="ExternalInput")
    b1sp_d = nc.dram_tensor("b1sp", [1152, BW], BF16, kind="ExternalInput")
    b1dp_d = nc.dram_tensor("b1dp", [1152, BW], BF16, kind="ExternalInput")
    mgap_d = nc.dram_tensor("mgap", [384, BW], BF16, kind="ExternalInput")
    msm_d = nc.dram_tensor("msm", [384, 384], BF16, kind="ExternalInput")
    mdf_d = nc.dram_tensor("mdf", [384, 384], BF16, kind="ExternalInput")
    mga_d = nc.dram_tensor("mga", [384, 384], BF16, kind="ExternalInput")
    s8_d = nc.dram_tensor("s8", [128, 16], BF16, kind="ExternalInput")
    t16_d = nc.dram_tensor("t16", [16, 128], BF16, kind="ExternalInput")
    id2_d = nc.dram_tensor("id2", [2, 2], F32, kind="ExternalInput")
    ninfh_d = nc.dram_tensor("ninfh", [128, 384], BF16, kind="ExternalInput")
    lw0_d = nc.dram_tensor("lw0", [2, 2], F32, kind="ExternalInput")
    e2b_d = nc.dram_tensor("e2b", [2, 128], F32, kind="ExternalInput")
    e64_d = nc.dram_tensor("e64", [128, 2], F32, kind="ExternalInput")
    iota128_d = nc.dram_tensor("iota128", [128, 1], F32, kind="ExternalInput")
    ones128_d = nc.dram_tensor("ones128", [128, 1], F32, kind="ExternalInput")
    ones1_d = nc.dram_tensor("ones1", [1, 128], F32, kind="ExternalInput")
    ones64h_d = nc.dram_tensor("ones64h", [1, 64], BF16, kind="ExternalInput")
    out_d = nc.dram_tensor("out", [4, 1], F32, kind="ExternalOutput")

    w1s, w1d, wga3 = C["w1s"], C["w1d"], C["wga3"]
    pr_df, pr_sm, pr_ga = C["pr_df"], C["pr_sm"], C["pr_ga"]

    with TileContext(nc) as tc:
        sb = tc.alloc_tile_pool(name="sb", bufs=1)
        sbi = tc.alloc_tile_pool(name="sbi", bufs=2)
        ps_cv = tc.alloc_tile_pool(name="pscv", bufs=2, space="PSUM")
        ps_g = tc.alloc_tile_pool(name="psg", bufs=2, space="PSUM")
        ps_m = tc.alloc_tile_pool(name="psm", bufs=2, space="PSUM")

        # ---- persistent SBUF ----
        b1sp_t = sb.tile([128, 9, BW], BF16)
        b1dp_t = sb.tile([128, 9, BW], BF16)
        mgap_t = sb.tile([128, 3, BW], BF16)
        msm_t = sb.tile([128, 3, 384], BF16)
        mdf_t = sb.tile([128, 3, 384], BF16)
        mga_t = sb.tile([128, 3, 384], BF16)
        s8_t = sb.tile([128, 16], BF16)
        t16_t = sb.tile([16, 128], BF16)
        id2_t = sb.tile([2, 2], F32)
        ninfh_t = sb.tile([128, 384], BF16)
        lw0_t = sb.tile([2, 2], F32)
        e2b_t = sb.tile([2, 128], F32)
        e64_t = sb.tile([128, 2], F32)
        iota128_t = sb.tile([128, 1], F32)
        ones128_t = sb.tile([128, 1], F32)
        ones1_t = sb.tile([1, 128], F32)
        ones64h_t = sb.tile([1, 64], BF16)

        spacc = sb.tile([128, 2], F32)     # softplus accums per image
        dacc = sb.tile([128, 6], F32)      # dot accums per (image, cc)
        gall = sb.tile([128, 4], F32)      # TTR accums: cross0, diag0, cross1, diag1
        xrow0_t = sb.tile([1, 2304], BF16)
        xrow1_t = sb.tile([1, 2304], BF16)
        xrow_tiles = [xrow0_t, xrow1_t]
        trash = sb.tile([128, 2304], BF16)
        trashf = sb.tile([128, 1216], F32)

        # colsum psum: [128, 512]: col = b*256 + t*16 + bi (diag at bi==t)
        cs = ps_m.tile([128, 512], F32, tag="cs", bufs=1)
        nc.vector.memset(cs, 0.0)

        # ---- input DMAs ----
        # img0 first on sync; desc on gpsimd (pool idle early)
        img_tiles, sd_tiles, dsl_tiles, dnm_tiles = [], [], [], []
        img0 = sbi.tile([128, 3, 3, 384], BF16, tag="img", bufs=2)
        nc.sync.dma_start(out=img0,
                          in_=imgs_d[0].rearrange("c (hc p) w -> p c hc w", p=128))
        nc.sync.dma_start(out=b1sp_t,
                          in_=b1sp_d[:, :].rearrange("(k p) h -> p k h", p=128))
        nc.sync.dma_start(out=b1dp_t,
                          in_=b1dp_d[:, :].rearrange("(k p) h -> p k h", p=128))
        nc.sync.dma_start(out=msm_t, in_=msm_d[:, :].rearrange("(k p) h -> p k h", p=128))
        nc.sync.dma_start(out=mdf_t, in_=mdf_d[:, :].rearrange("(k p) h -> p k h", p=128))
        nc.sync.dma_start(out=mgap_t, in_=mgap_d[:, :].rearrange("(k p) h -> p k h", p=128))
        nc.sync.dma_start(out=mga_t, in_=mga_d[:, :].rearrange("(k p) h -> p k h", p=128))
        for b in range(NIMG):
            dnm = sbi.tile([128, 16, DDIM], FP8, tag="dnm", bufs=2)
            nc.gpsimd.dma_start(out=dnm, in_=dnm_d[b])
            dnm_tiles.append(dnm)
            dsl = sbi.tile([128, 2, NDESC], FP8, tag="dsl", bufs=2)
            nc.gpsimd.dma_start(out=dsl, in_=dsl_d[b])
            dsl_tiles.append(dsl)
        for b in range(NIMG):
            sdt = sbi.tile([128, 3, 384], BF16, tag="sdt", bufs=2)
            nc.gpsimd.dma_start(out=sdt,
                                in_=sd_d[b].rearrange("(c p) w -> p c w", p=128))
            sd_tiles.append(sdt)
        img1 = sbi.tile([128, 3, 3, 384], BF16, tag="img", bufs=2)
        nc.sync.dma_start(out=img1,
                          in_=imgs_d[1].rearrange("c (hc p) w -> p c hc w", p=128))
        img_tiles.extend([img0, img1])
        for t, d in [(s8_t, s8_d), (t16_t, t16_d), (id2_t, id2_d), (ninfh_t, ninfh_d),
                     (lw0_t, lw0_d), (e2b_t, e2b_d), (e64_t, e64_d),
                     (iota128_t, iota128_d), (ones128_t, ones128_d), (ones1_t, ones1_d),
                     (ones64h_t, ones64h_d)]:
            nc.sync.dma_start(out=t, in_=d[:, :])

        # ---- r = rsqrt(nsq) from n-major fp8: wide square + wide reduce ----
        r_tiles = []
        dsq = sb.tile([128, 16, DDIM], BF16)

        def emit_r(b):
            dnm = dnm_tiles[b]
            nc.scalar.activation(dsq, dnm, AF.Square)
            nsqf = sbi.tile([128, 16], F32, tag="nsqf", bufs=2)
            nc.vector.tensor_reduce(nsqf, dsq, axis=AX.X, op=ALU.add)
            sr = sbi.tile([128, 16], F32, tag="sr", bufs=2)
            nc.scalar.activation(sr, nsqf, AF.Sqrt)
            y0 = sbi.tile([128, 16], F32, tag="y0", bufs=2)
            nc.vector.reciprocal(y0, sr)
            yy = sbi.tile([128, 16], F32, tag="yy", bufs=2)
            nc.vector.tensor_tensor(out=yy, in0=y0, in1=y0, op=ALU.mult)
            nc.vector.tensor_tensor(out=yy, in0=yy, in1=nsqf, op=ALU.mult)
            nc.vector.tensor_scalar(yy, yy, -0.5, 1.5, op0=ALU.mult, op1=ALU.add)
            r_bf = sbi.tile([128, 16], BF16, tag="rbf", bufs=2)
            nc.vector.tensor_tensor(out=r_bf, in0=yy, in1=y0, op=ALU.mult)
            r_tiles.append(r_bf)

        # ---- gram tile generator ----
        def gram_tiles_gen():
            for b in range(NIMG):
                for bi in range(16):
                    c0 = 128 * bi
                    pos = c0
                    while pos < NDESC:
                        wdt = min(1024, NDESC - pos)
                        yield (b, bi, pos, wdt)
                        pos += wdt

        _gram_iter = gram_tiles_gen()
        _gram_state = {"done": False, "acc": 0.0, "pending": []}

        def _emit_matvecs(ent):
            grelu, b, bi, pos, wdt = ent
            r_bf = r_tiles[b]
            for ci in range(wdt // 128):
                t = (pos + ci * 128) // 128
                col = 256 * b + 16 * t + bi
                nc.tensor.matmul(cs[:, col:col + 1],
                                 grelu[:, 128 * ci:128 * (ci + 1)],
                                 r_bf[:, bi:bi + 1],
                                 start=True, stop=True)

        def pump_gram(n, act_share=0.0):
            for _ in range(n):
                if _gram_state["done"]:
                    break
                try:
                    b, bi, pos, wdt = next(_gram_iter)
                except StopIteration:
                    _gram_state["done"] = True
                    break
                dsl = dsl_tiles[b]
                gp = ps_g.tile([128, 1024], F32, tag="g")
                off = 0
                while off < wdt:
                    nn = min(512, wdt - off)
                    nc.tensor.matmul(gp[:, off:off + nn],
                                     dsl[:, :, 128 * bi:128 * (bi + 1)],
                                     dsl[:, :, pos + off:pos + off + nn],
                                     start=True, stop=True, perf_mode=DR)
                    off += nn
                grelu = sbi.tile([128, 1024], BF16, tag="grelu", bufs=5)
                _gram_state["acc"] += act_share
                if _gram_state["acc"] >= 1.0:
                    _gram_state["acc"] -= 1.0
                    nc.scalar.activation(grelu[:, 0:wdt], gp[:, 0:wdt], AF.Relu)
                else:
                    nc.vector.tensor_scalar(grelu[:, 0:wdt], gp[:, 0:wdt], 0.0,
                                            None, op0=ALU.max)
                _gram_state["pending"].append((grelu, b, bi, pos, wdt))
                while len(_gram_state["pending"]) > 3:
                    _emit_matvecs(_gram_state["pending"].pop(0))
            if _gram_state["done"]:
                while _gram_state["pending"]:
                    _emit_matvecs(_gram_state["pending"].pop(0))

        # ---- conv + NMS per image ----
        resp_list, nms_list, bw_list = [], [], []

        def emit_conv(b, pump=True):
            img_t = img_tiles[b]
            imgv = img_t.rearrange("p c hc w -> p (c hc) w")

            # P1: smooth/diff along H -> [w-part, wb, h]
            sT = sbi.tile([128, 3, 384], BF16, tag="sT")
            dT = sbi.tile([128, 3, 384], BF16, tag="dT")
            for di, (dst, bnd, wins) in enumerate(
                    ((sT, b1sp_t, w1s), (dT, b1dp_t, w1d))):
                for wb in range(3):
                    pst = ps_cv.tile([128, 384], F32, tag="cv")
                    first = True
                    for k in range(9):
                        if wins[k] is None:
                            continue
                        c0, c1 = wins[k]
                        nc.tensor.matmul(pst[:, c0:c1],
                                         imgv[:, k, wb * 128:(wb + 1) * 128],
                                         bnd[:, k, 0:c1 - c0], start=first, stop=False)
                        first = False
                    if (di * 3 + wb) % 2 == 0:
                        nc.scalar.copy(dst[:, wb, :], pst)
                    else:
                        nc.vector.tensor_copy(dst[:, wb, :], pst)

            # P2: diff/smooth along W -> px=dx, py=dy [w-part(ob), h]
            pq = sbi.tile([128, 3, 384], BF16, tag="pq", bufs=1)
            qq = sbi.tile([128, 3, 384], BF16, tag="qq", bufs=1)
            rr = sbi.tile([128, 3, 384], BF16, tag="rr", bufs=1)
            dxs = sbi.tile([128, 384], BF16, tag="dxs", bufs=2)
            for ob in range(3):
                px = ps_cv.tile([128, 384], F32, tag="cv")
                fx = True
                for kc, ob2 in pr_df:
                    if ob2 != ob:
                        continue
                    nc.tensor.matmul(px, mdf_t[:, kc, ob * 128:(ob + 1) * 128],
                                     sT[:, kc, :], start=fx, stop=False)
                    fx = False
                nc.scalar.activation(pq[:, ob, :], px, AF.Square)
                nc.scalar.copy(dxs, px)
                py = ps_cv.tile([128, 384], F32, tag="cv")
                fy = True
                for kc, ob2 in pr_sm:
                    if ob2 != ob:
                        continue
                    nc.tensor.matmul(py, msm_t[:, kc, ob * 128:(ob + 1) * 128],
                                     dT[:, kc, :], start=fy, stop=False)
                    fy = False
                nc.scalar.activation(qq[:, ob, :], py, AF.Square)
                nc.vector.tensor_tensor(out=rr[:, ob, :], in0=dxs, in1=py, op=ALU.mult)
            # plus/minus (linear through gauss)
            pl = sbi.tile([128, 3, 384], BF16, tag="pl", bufs=1)
            mi = sbi.tile([128, 3, 384], BF16, tag="mi", bufs=1)
            nc.vector.tensor_tensor(out=pl, in0=pq, in1=qq, op=ALU.add)
            nc.vector.tensor_tensor(out=mi, in0=pq, in1=qq, op=ALU.subtract)

            if pump:
                pump_gram(2, act_share=0.2)

            # P3: gauss along W -> [h-part(hb), w]
            gP = sbi.tile([128, 3, 384], BF16, tag="gP", bufs=1)
            gM = sbi.tile([128, 3, 384], BF16, tag="gM", bufs=1)
            gR = sbi.tile([128, 3, 384], BF16, tag="gR", bufs=1)
            for si, (src, dst) in enumerate(((pl, gP), (mi, gM), (rr, gR))):
                for hb in range(3):
                    pst = ps_cv.tile([128, 384], F32, tag="cv")
                    for i, cw in enumerate(range(3)):
                        c0, c1 = wga3[cw]
                        nc.tensor.matmul(pst[:, c0:c1],
                                         src[:, cw, hb * 128:(hb + 1) * 128],
                                         mgap_t[:, cw, 0:c1 - c0],
                                         start=(i == 0), stop=False)
                    if (si * 3 + hb) % 2 == 0:
                        nc.scalar.copy(dst[:, hb, :], pst)
                    else:
                        nc.vector.tensor_copy(dst[:, hb, :], pst)

            if pump:
                pump_gram(2, act_share=0.2)

            # P4: gauss along H -> P (tr), M, R in [h-part(ob), w]; response
            resp = sbi.tile([128, 3, 388], BF16, tag="resp", bufs=2)
            for ob in range(3):
                pR = ps_cv.tile([128, 384], F32, tag="cv")
                first = True
                for kc, ob2 in pr_ga:
                    if ob2 != ob:
                        continue
                    nc.tensor.matmul(pR, mga_t[:, kc, ob * 128:(ob + 1) * 128],
                                     gR[:, kc, :], start=first, stop=False)
                    first = False
                B4 = sbi.tile([128, 384], BF16, tag="B4", bufs=2)
                nc.scalar.activation(B4, pR, AF.Square, scale=2.0)
                pM = ps_cv.tile([128, 384], F32, tag="cv")
                first = True
                for kc, ob2 in pr_ga:
                    if ob2 != ob:
                        continue
                    nc.tensor.matmul(pM, mga_t[:, kc, ob * 128:(ob + 1) * 128],
                                     gM[:, kc, :], start=first, stop=False)
                    first = False
                A2 = sbi.tile([128, 384], BF16, tag="A2", bufs=2)
                nc.scalar.activation(A2, pM, AF.Square)
                disc = sbi.tile([128, 384], BF16, tag="disc", bufs=2)
                nc.vector.tensor_tensor(out=disc, in0=A2, in1=B4, op=ALU.add)
                s2 = sbi.tile([128, 384], F32, tag="s2", bufs=2)
                nc.scalar.activation(s2, disc, AF.Sqrt, scale=0.25)
                pP = ps_cv.tile([128, 384], F32, tag="cv")
                first = True
                for kc, ob2 in pr_ga:
                    if ob2 != ob:
                        continue
                    nc.tensor.matmul(pP, mga_t[:, kc, ob * 128:(ob + 1) * 128],
                                     gP[:, kc, :], start=first, stop=False)
                    first = False
                nc.vector.scalar_tensor_tensor(out=resp[:, ob, 2:386], in0=pP,
                                               scalar=0.5, in1=s2,
                                               op0=ALU.mult, op1=ALU.subtract)
                nc.vector.tensor_copy(resp[:, ob, 0:2], ninfh_t[:, 0:2])
                nc.vector.tensor_copy(resp[:, ob, 386:388], ninfh_t[:, 0:2])
            resp_list.append(resp)

        def emit_nms_gen(b):
            TTs = nc.vector.tensor_tensor
            resp = resp_list[b]
            sdt = sd_tiles[b]
            sdv = sdt.rearrange("p c w -> p (c w)")
            spA = sbi.tile([128, 1152], F32, tag="spA", bufs=1)
            nc.scalar.activation(spA, sdv, AF.Exp)
            nc.scalar.activation(trashf[:, 0:1152], spA, AF.Ln, bias=1.0,
                                 accum_out=spacc[:, b:b + 1])

            t1 = sbi.tile([128, 3, 388], BF16, tag="t1", bufs=1)
            TTs(out=t1[:, :, 0:387], in0=resp[:, :, 0:387],
                in1=resp[:, :, 1:388], op=ALU.max)
            t2 = sbi.tile([128, 3, 388], BF16, tag="t2", bufs=1)
            TTs(out=t2[:, :, 0:385], in0=t1[:, :, 0:385],
                in1=t1[:, :, 2:387], op=ALU.max)
            m1 = sbi.tile([128, 3, 384], BF16, tag="m1", bufs=2)
            TTs(out=m1, in0=t2[:, :, 0:384], in1=resp[:, :, 4:388], op=ALU.max)
            yield
            shs = []
            for k in (1, 2):
                sh = sbi.tile([128, 3, 384], BF16, tag="shd", bufs=2)
                nc.vector.memset(sh[0:k, 0, :], NEG)
                nc.sync.dma_start(out=sh[k:128], in_=m1[0:128 - k])
                nc.sync.dma_start(out=sh[0:k, 1:3, :], in_=m1[128 - k:128, 0:2, :])
                shs.append(sh)
                sh2 = sbi.tile([128, 3, 384], BF16, tag="shu", bufs=2)
                nc.gpsimd.dma_start(out=sh2[128 - k:128, 2, :],
                                    in_=ninfh_t[0:k, 0:384])
                nc.gpsimd.dma_start(out=sh2[0:128 - k], in_=m1[k:128])
                nc.gpsimd.dma_start(out=sh2[128 - k:128, 0:2, :], in_=m1[0:k, 1:3, :])
                shs.append(sh2)
            yield
            mp1 = sbi.tile([128, 3, 384], BF16, tag="mp1", bufs=1)
            nc.vector.tensor_tensor(out=mp1, in0=m1, in1=shs[0], op=ALU.max)
            mp2 = sbi.tile([128, 3, 384], BF16, tag="mp2", bufs=1)
            TTs(out=mp2, in0=shs[1], in1=shs[2], op=ALU.max)
            nc.vector.tensor_tensor(out=mp2, in0=mp2, in1=shs[3], op=ALU.max)
            mp = sbi.tile([128, 3, 384], BF16, tag="mp", bufs=2)
            nc.vector.tensor_tensor(out=mp, in0=mp1, in1=mp2, op=ALU.max)
            yield
            e1 = sbi.tile([128, 3, 384], BF16, tag="e1", bufs=1)
            TTs(out=e1, in0=resp[:, :, 2:386], in1=mp, op=ALU.is_ge)
            nms = sbi.tile([128, 3, 384], BF16, tag="nms", bufs=2)
            nc.vector.tensor_tensor(out=nms, in0=resp[:, :, 2:386], in1=e1, op=ALU.mult)
            nms_list.append(nms)
            yield
            bw = sbi.tile([128, 3, 48], BF16, tag="bw")
            nc.vector.tensor_reduce(bw, nms.rearrange("p c (g k) -> p c g k", k=8),
                                    axis=AX.X, op=ALU.max)
            sh_1 = sbi.tile([128, 3, 48], BF16, tag="shb", bufs=2)
            nc.sync.dma_start(out=sh_1[0:127], in_=bw[1:128])
            nc.sync.dma_start(out=sh_1[127:128], in_=bw[127:128])
            sh_2 = sbi.tile([128, 3, 48], BF16, tag="shb", bufs=2)
            nc.gpsimd.dma_start(out=sh_2[0:126], in_=bw[2:128])
            nc.gpsimd.dma_start(out=sh_2[126:128], in_=bw[126:128])
            sh_3 = sbi.tile([128, 3, 48], BF16, tag="shb", bufs=2)
            nc.sync.dma_start(out=sh_3[0:125], in_=bw[3:128])
            nc.sync.dma_start(out=sh_3[125:128], in_=bw[125:128])
            yield
            q1 = sbi.tile([128, 3, 48], BF16, tag="bwm", bufs=2)
            nc.vector.tensor_tensor(out=q1, in0=bw, in1=sh_1, op=ALU.max)
            q2 = sbi.tile([128, 3, 48], BF16, tag="bwm", bufs=2)
            TTs(out=q2, in0=sh_2, in1=sh_3, op=ALU.max)
            q3 = sbi.tile([128, 3, 48], BF16, tag="bwm3", bufs=2)
            nc.vector.tensor_tensor(out=q3, in0=q1, in1=q2, op=ALU.max)
            sh_4 = sbi.tile([128, 3, 48], BF16, tag="shb", bufs=2)
            nc.sync.dma_start(out=sh_4[0:124], in_=q3[4:128])
            nc.sync.dma_start(out=sh_4[124:128], in_=q3[124:128])
            yield
            cur = sbi.tile([128, 3, 48], BF16, tag="bwf", bufs=2)
            nc.vector.tensor_tensor(out=cur, in0=q3, in1=sh_4, op=ALU.max)
            yield
            p16 = ps_m.tile([16, 144], F32, tag="m", bufs=1)
            nc.tensor.matmul(p16, s8_t, cur.rearrange("p c g -> p (c g)"),
                             start=True, stop=True)
            p16s = sbi.tile([16, 3, 48], BF16, tag="p16s", bufs=2)
            nc.scalar.copy(p16s.rearrange("p c g -> p (c g)"), p16)
            bw_list.append(p16s)
            xr = xrow_tiles[b]
            nc.sync.dma_start(out=xr, in_=p16s.rearrange("p c g -> p (c g)"))

        # ---- threshold search + selection, as an interleavable generator ----
        def thresh_select_gen():
            xrow0, xrow1 = xrow_tiles
            x128 = sb.tile([128, 2304], BF16)
            nc.gpsimd.partition_broadcast(x128[0:64], xrow0, channels=64)
            yield
            for ci, off in enumerate(range(0, 2304, 512)):
                nn = min(512, 2304 - off)
                bps = ps_m.tile([128, 512], F32, tag="m", bufs=1)
                nc.tensor.matmul(bps[64:128, 0:nn], ones64h_t,
                                 xrow1[0:1, off:off + nn], start=True, stop=False,
                                 tile_position=(0, 64))
                nc.scalar.copy(x128[64:128, off:off + nn], bps[64:128, 0:nn])
            yield

            lw_t = sb.tile([2, 2], F32)
            nc.vector.tensor_copy(lw_t, lw0_t)
            for rnd in range(2):
                if rnd == 0:
                    T_t = sb.tile([128, 1], F32, tag="Tthr", bufs=2)
                    nc.vector.tensor_scalar(T_t, iota128_t, 0.25 / 64.0, None,
                                            op0=ALU.mult)
                else:
                    lwb_ps = ps_m.tile([128, 2], F32, tag="m", bufs=1)
                    nc.tensor.matmul(lwb_ps, e2b_t, lw_t, start=True, stop=True)
                    lwb = sb.tile([128, 2], F32, tag="lwb", bufs=2)
                    nc.scalar.copy(lwb, lwb_ps)
                    T_t = sb.tile([128, 1], F32, tag="Tthr", bufs=2)
                    nc.vector.tensor_scalar(T_t, iota128_t, lwb[:, 1:2], lwb[:, 0:1],
                                            op0=ALU.mult, op1=ALU.add)
                yield
                cnt128 = sb.tile([128, 1], F32, tag="cnt", bufs=2)
                nc.vector.tensor_scalar(trash, x128, T_t[:, 0:1], None,
                                        op0=ALU.is_gt, op1=ALU.add,
                                        accum_out=cnt128)
                mask = sb.tile([128, 1], F32, tag="mask", bufs=2)
                nc.vector.tensor_scalar(mask, cnt128, float(NUM) - 0.5, None,
                                        op0=ALU.is_ge)
                kps = ps_m.tile([2, 1], F32, tag="m", bufs=1)
                nc.tensor.matmul(kps, e64_t, mask, start=True, stop=True)
                yield
                t1k = sb.tile([2, 1], F32, tag="tk", bufs=2)
                nc.vector.tensor_scalar(t1k, kps, lw_t[:, 1:2], lw_t[:, 0:1],
                                        op0=ALU.mult, op1=ALU.add)
                nc.vector.tensor_scalar(lw_t[:, 0:1], t1k, lw_t[:, 1:2], 0.0,
                                        op0=ALU.subtract, op1=ALU.max)
                if rnd < 1:
                    nc.vector.tensor_scalar(lw_t[:, 1:2], lw_t[:, 1:2], 1.0 / 64.0,
                                            None, op0=ALU.mult)
                yield
            lo_t = sb.tile([2, 1], F32)
            nc.vector.tensor_scalar(lo_t, lw_t[:, 0:1], 1e-30, None, op0=ALU.max)
            tbrp = ps_m.tile([1, 2], F32, tag="m", bufs=1)
            nc.tensor.matmul(tbrp, lo_t, id2_t, start=True, stop=True)
            tbr = sb.tile([1, 2], F32)
            nc.scalar.copy(tbr, tbrp)
            tbcp = ps_m.tile([128, 2], F32, tag="m", bufs=1)
            nc.tensor.matmul(tbcp, ones1_t, tbr, start=True, stop=True)
            tbc = sb.tile([128, 2], F32)
            nc.scalar.copy(tbc, tbcp)
            yield
            # selection + dot per image
            for b in range(NIMG):
                p16s = bw_list[b]
                nms = nms_list[b]
                sdt = sd_tiles[b]
                p16c = sbi.tile([16, 3, 48], BF16, tag="p16c", bufs=2)
                nc.vector.tensor_scalar(p16c.rearrange("p c g -> p (c g)"),
                                        p16s.rearrange("p c g -> p (c g)"),
                                        tbc[0:16, b:b + 1], None, op0=ALU.max)
                yield
                for cc in range(3):
                    bexp = ps_m.tile([128, 384], F32, tag="m", bufs=1)
                    nc.tensor.matmul(bexp, t16_t,
                                     p16c[:, cc, :].unsqueeze(2)
                                     .to_broadcast([16, 48, 8]),
                                     start=True, stop=True)
                    sel = sbi.tile([128, 384], BF16, tag="sel", bufs=2)
                    nc.vector.tensor_tensor(out=sel, in0=nms[:, cc, :], in1=bexp,
                                            op=ALU.is_ge)
                    dtmp = sbi.tile([128, 384], BF16, tag="dtmp", bufs=2)
                    nc.vector.tensor_tensor(out=dtmp, in0=sel, in1=sdt[:, cc, :],
                                            op=ALU.mult)
                    nc.vector.tensor_reduce(dacc[:, 3 * b + cc:3 * b + cc + 1],
                                            dtmp, axis=AX.X, op=ALU.add)
                    yield

        # ================= schedule =================
        def drive(gen, tiles_per_step=1, act_share=0.45):
            for _ in gen:
                pump_gram(tiles_per_step, act_share=act_share)

        emit_conv(0, pump=False)
        pump_gram(3)
        emit_r(0)
        emit_r(1)
        drive(emit_nms_gen(0), 1, 0.3)
        emit_conv(1)
        drive(emit_nms_gen(1), 2, 0.45)
        pump_gram(1000, act_share=0.45)
        for _ in thresh_select_gen():
            pass

        # gram weighted sums per image: full (incl diag) and diag-only
        for b in range(NIMG):
            wcs = sbi.tile([128, 256], F32, tag="wcs", bufs=2)
            nc.vector.tensor_tensor(
                out=wcs.rearrange("p (a k) -> p a k", k=16),
                in0=cs[:, 256 * b:256 * b + 256].rearrange("p (a k) -> p a k", k=16),
                in1=r_tiles[b].unsqueeze(2).to_broadcast([128, 16, 16]),
                op=ALU.mult)
            nc.vector.tensor_reduce(gall[:, 2 * b:2 * b + 1], wcs,
                                    axis=AX.X, op=ALU.add)
            wcd = sbi.tile([128, 16], F32, tag="wcd", bufs=2)
            import concourse.ap as ap_mod
            csap = cs[:, 256 * b:256 * b + 256]
            diag_ap = ap_mod.AP(csap.tensor, csap.offset,
                                [list(csap.ap[0]), [17 * csap.ap[1][0], 16]])
            nc.vector.tensor_tensor(out=wcd, in0=diag_ap, in1=r_tiles[b],
                                    op=ALU.mult)
            nc.vector.tensor_reduce(gall[:, 2 * b + 1:2 * b + 2], wcd,
                                    axis=AX.X, op=ALU.add)

        # ---- final reduce ----
        vals = sb.tile([128, 4], F32)
        nc.vector.tensor_reduce(vals[:, 0:1], spacc, axis=AX.X, op=ALU.add)
        nc.vector.tensor_reduce(vals[:, 1:2], dacc, axis=AX.X, op=ALU.add)
        nc.vector.tensor_tensor(out=vals[:, 2:3], in0=gall[:, 0:1], in1=gall[:, 2:3],
                                op=ALU.add)
        nc.vector.tensor_tensor(out=vals[:, 3:4], in0=gall[:, 1:2], in1=gall[:, 3:4],
                                op=ALU.add)
        fps = ps_m.tile([4, 1], F32, tag="m", bufs=1)
        nc.tensor.matmul(fps, vals, ones128_t, start=True, stop=True)
        fsb = sb.tile([4, 1], F32)
        nc.scalar.copy(fsb, fps)
        nc.sync.dma_start(out=out_d[:, :], in_=fsb)

        ps_m.release()
        ps_g.release()
        ps_cv.release()
        sbi.release()
        sb.release()

    nc.finalize()
    return nc, C


_CACHE = {}


def kernel(descriptors, scores, scores_dense, imgs):
    B = descriptors.shape[0]
    ncore = 8
    per = B // ncore
    if "nc" not in _CACHE:
        _CACHE["nc"], _CACHE["C"] = build_program()
    nc, C = _CACHE["nc"], _CACHE["C"]

    imgs_bf = np.ascontiguousarray(np.asarray(imgs).astype(ml_dtypes.bfloat16))
    sd = np.ascontiguousarray(np.asarray(scores_dense).reshape(B, H, W)
                              .astype(ml_dtypes.bfloat16))
    desc8 = np.asarray(descriptors).astype(ml_dtypes.float8_e4m3)
    # slab d-major: [B, 128(dj), 2(slab), N]
    dsl = np.ascontiguousarray(desc8.transpose(0, 2, 1)
                               .reshape(B, 2, 128, NDESC).transpose(0, 2, 1, 3))
    # n-major: [B, 128(n in chunk), 16(chunk), D]
    dnm = np.ascontiguousarray(desc8.reshape(B, 16, 128, DDIM).transpose(0, 2, 1, 3))

    in_maps = []
    for c in range(ncore):
        s = slice(c * per, (c + 1) * per)
        in_maps.append({
            "imgs": imgs_bf[s], "sd": sd[s], "dsl": dsl[s], "dnm": dnm[s],
            "b1sp": C["b1sp"], "b1dp": C["b1dp"], "mgap": C["mgap"],
            "msm": C["msm"], "mdf": C["mdf"], "mga": C["mga"],
            "s8": C["s8"], "t16": C["t16"], "id2": C["id2"], "ninfh": C["ninfh"],
            "lw0": C["lw0"], "e2b": C["e2b"], "e64": C["e64"],
            "iota128": C["iota128"], "ones128": C["ones128"], "ones1": C["ones1"],
            "ones64h": C["ones64h"],
        })

    res = run_bass_kernel_spmd(nc, in_maps, core_ids=list(range(ncore)))
    S1 = S2 = Sall = Sdia = 0.0
    for c in range(ncore):
        o = np.asarray(res.results[c]["out"])[:, 0].astype(np.float64)
        S1 += o[0]
        S2 += o[1]
        Sall += o[2]
        Sdia += o[3]
    bce = (S1 - S2) / (B * H * W)
    relu_mean = (2.0 * Sall - Sdia) / (B * NDESC * NDESC)
    return np.array(bce + relu_mean, dtype=np.float32)


# revision 5
# speedup vs baseline: 1.1098x; 1.1098x over previous
"""Trainium2 Bass kernel for nn_DistinctionLoss (GFTT corners BCE + relu-cosine mean).

v2: batch-sharded 2 images/core across 8 cores.
 - fp8 DoubleRow raw gram (host-side e4m3 cast + d-major slab layout);
   normalization folded in post-relu via N=1 PE matvecs with r = rsqrt(diag).
 - GFTT restructured: (dx^2+dy^2, dx^2-dy^2) pushed through the linear gaussian
   convs, deleting the tr/A add/sub stages.
 - Elementwise spread across Act/DVE/Pool; bf16 everywhere DVE gets 2x mode.
"""
import os
import numpy as np
import ml_dtypes

import concourse.bacc as bacc
import concourse.mybir as mybir
from concourse.tile import TileContext
from concourse.bass_utils import run_bass_kernel_spmd

F32 = mybir.dt.float32
BF16 = mybir.dt.bfloat16
FP8 = mybir.dt.float8e4
AF = mybir.ActivationFunctionType
ALU = mybir.AluOpType
AX = mybir.AxisListType
DR = mybir.MatmulPerfMode.DoubleRow

H = W = 384
NIMG = 2
NDESC = 2048
DDIM = 256
NUM = 200
NEG = -1e30
BW = 136  # packed band window width

_bf = lambda a: np.ascontiguousarray(a.astype(ml_dtypes.bfloat16))


def _band(k, mode, n=384):
    pad = len(k) // 2
    idx = np.arange(n + 2 * pad) - pad
    if mode == "edge":
        src = np.clip(idx, 0, n - 1)
    else:  # reflect
        src = np.abs(idx)
        src = np.where(src >= n, 2 * (n - 1) - src, src)
    M = np.zeros((n, n), np.float32)
    for i, kv in enumerate(k):
        M[src[np.arange(n) + i], np.arange(n)] += kv
    return M


def _gauss7():
    xs = np.arange(7, dtype=np.float32) - 3.0
    g = np.exp(-0.5 * xs ** 2)
    return (g / g.sum()).astype(np.float32)


def _wins(M, nchunk):
    wins = []
    for k in range(nchunk):
        rows = M[k * 128:(k + 1) * 128]
        nz = np.nonzero(np.any(rows != 0, axis=0))[0]
        wins.append((int(nz[0]), int(nz[-1]) + 1) if len(nz) else None)
    return wins


def _pack(M, nchunk, wins):
    """Pack band matrix rows into [nchunk*128, BW] windows."""
    P = np.zeros((nchunk * 128, BW), np.float32)
    for k in range(nchunk):
        if wins[k] is None:
            continue
        c0, c1 = wins[k]
        P[k * 128:(k + 1) * 128, 0:c1 - c0] = M[k * 128:(k + 1) * 128, c0:c1]
    return P


def _nzpairs(M):
    out = []
    for ob in range(3):
        for kc in range(3):
            if np.any(M[kc * 128:(kc + 1) * 128, ob * 128:(ob + 1) * 128]):
                out.append((kc, ob))
    return out


def _consts():
    c = {}
    Msm = _band(np.array([1, 2, 1], np.float32) / 8.0, "edge")
    Mdf = _band(np.array([-1, 0, 1], np.float32), "edge")
    Mga = _band(_gauss7(), "reflect")
    coef = np.array([0.299, 0.587, 0.114], np.float32)
    b1s = np.concatenate([coef[i] * Msm for i in range(3)], axis=0)
    b1d = np.concatenate([coef[i] * Mdf for i in range(3)], axis=0)
    c["w1s"] = _wins(b1s, 9)
    c["w1d"] = _wins(b1d, 9)
    c["wga3"] = _wins(Mga, 3)
    c["pr_df"] = _nzpairs(Mdf)
    c["pr_sm"] = _nzpairs(Msm)
    c["pr_ga"] = _nzpairs(Mga)
    c["b1sp"] = _bf(_pack(b1s, 9, c["w1s"]))
    c["b1dp"] = _bf(_pack(b1d, 9, c["w1d"]))
    c["mgap"] = _bf(_pack(Mga, 3, c["wga3"]))
    c["msm"] = _bf(Msm)
    c["mdf"] = _bf(Mdf)
    c["mga"] = _bf(Mga)
    S8 = np.zeros((128, 16), np.float32)
    S8[np.arange(16) * 8, np.arange(16)] = 1.0
    c["s8"] = _bf(S8)
    T16 = np.zeros((16, 128), np.float32)
    T16[np.arange(128) // 8, np.arange(128)] = 1.0
    c["t16"] = _bf(T16)
    c["id2"] = np.eye(2, dtype=np.float32)
    c["ninfh"] = _bf(np.full((128, 384), NEG, np.float32))
    c["lw0"] = np.array([[0.0, 0.25 / 64.0], [0.0, 0.25 / 64.0]], np.float32)
    halves = np.zeros((2, 128), np.float32)
    halves[0, :64] = 1.0
    halves[1, 64:] = 1.0
    c["e2b"] = halves.copy()
    c["e64"] = np.ascontiguousarray(halves.T)
    c["iota128"] = (np.arange(128, dtype=np.float32) % 64).reshape(128, 1)
    c["thrW0"] = (np.arange(64, dtype=np.float32) * (0.25 / 64.0)).reshape(64, 1)
    c["thrW1"] = (np.arange(64, dtype=np.float32) * (0.25 / 4096.0)).reshape(64, 1)
    c["ones128"] = np.ones((128, 1), np.float32)
    c["ones64h"] = _bf(np.ones((1, 64), np.float32))
    c["ones1"] = np.ones((1, 128), np.float32)
    return c


def build_program():
    C = _consts()
    nc = bacc.Bacc()

    imgs_d = nc.dram_tensor("imgs", [NIMG, 3, H, W], BF16, kind="ExternalInput")
    sd_d = nc.dram_tensor("sd", [NIMG, H, W], BF16, kind="ExternalInput")
    dsl_d = nc.dram_tensor("dsl", [NIMG, 128, 2, NDESC], FP8, kind="ExternalInput")
    dnm_d = nc.dram_tensor("dnm", [NIMG, 128, 16, DDIM], FP8, kind="ExternalInput")
    b1sp_d = nc.dram_tensor("b1sp", [1152, BW], BF16, kind="ExternalInput")
    b1dp_d = nc.dram_tensor("b1dp", [1152, BW], BF16, kind="ExternalInput")
    mgap_d = nc.dram_tensor("mgap", [384, BW], BF16, kind="ExternalInput")
    msm_d = nc.dram_tensor("msm", [384, 384], BF16, kind="ExternalInput")
    mdf_d = nc.dram_tensor("mdf", [384, 384], BF16, kind="ExternalInput")
    mga_d = nc.dram_tensor("mga", [384, 384], BF16, kind="ExternalInput")
    s8_d = nc.dram_tensor("s8", [128, 16], BF16, kind="ExternalInput")
    t16_d = nc.dram_tensor("t16", [16, 128], BF16, kind="ExternalInput")
    id2_d = nc.dram_tensor("id2", [2, 2], F32, kind="ExternalInput")
    ninfh_d = nc.dram_tensor("ninfh", [128, 384], BF16, kind="ExternalInput")
    lw0_d = nc.dram_tensor("lw0", [2, 2], F32, kind="ExternalInput")
    e2b_d = nc.dram_tensor("e2b", [2, 128], F32, kind="ExternalInput")
    e64_d = nc.dram_tensor("e64", [128, 2], F32, kind="ExternalInput")
    iota128_d = nc.dram_tensor("iota128", [128, 1], F32, kind="ExternalInput")
    thrW0_d = nc.dram_tensor("thrW0", [64, 1], F32, kind="ExternalInput")
    thrW1_d = nc.dram_tensor("thrW1", [64, 1], F32, kind="ExternalInput")
    ones128_d = nc.dram_tensor("ones128", [128, 1], F32, kind="ExternalInput")
    ones1_d = nc.dram_tensor("ones1", [1, 128], F32, kind="ExternalInput")
    ones64h_d = nc.dram_tensor("ones64h", [1, 64], BF16, kind="ExternalInput")
    out_d = nc.dram_tensor("out", [4, 1], F32, kind="ExternalOutput")

    w1s, w1d, wga3 = C["w1s"], C["w1d"], C["wga3"]
    pr_df, pr_sm, pr_ga = C["pr_df"], C["pr_sm"], C["pr_ga"]

    with TileContext(nc) as tc:
        sb = tc.alloc_tile_pool(name="sb", bufs=1)
        sbi = tc.alloc_tile_pool(name="sbi", bufs=2)
        ps_cv = tc.alloc_tile_pool(name="pscv", bufs=2, space="PSUM")
        ps_g = tc.alloc_tile_pool(name="psg", bufs=2, space="PSUM")
        ps_m = tc.alloc_tile_pool(name="psm", bufs=2, space="PSUM")

        # ---- persistent SBUF ----
        b1sp_t = sb.tile([128, 9, BW], BF16)
        b1dp_t = sb.tile([128, 9, BW], BF16)
        mgap_t = sb.tile([128, 3, BW], BF16)
        msm_t = sb.tile([128, 3, 384], BF16)
        mdf_t = sb.tile([128, 3, 384], BF16)
        mga_t = sb.tile([128, 3, 384], BF16)
        s8_t = sb.tile([128, 16], BF16)
        t16_t = sb.tile([16, 128], BF16)
        id2_t = sb.tile([2, 2], F32)
        ninfh_t = sb.tile([128, 384], BF16)
        lw0_t = sb.tile([2, 2], F32)
        e2b_t = sb.tile([2, 128], F32)
        e64_t = sb.tile([128, 2], F32)
        iota128_t = sb.tile([128, 1], F32)
        thrW0_t = sb.tile([64, 1], F32)
        thrW1_t = sb.tile([64, 1], F32)
        ones128_t = sb.tile([128, 1], F32)
        ones1_t = sb.tile([1, 128], F32)
        ones64h_t = sb.tile([1, 64], BF16)

        spacc = sb.tile([128, 2], F32)     # softplus accums per image
        dacc = sb.tile([128, 6], F32)      # dot accums per (image, cc)
        gall = sb.tile([128, 4], F32)      # TTR accums: cross0, diag0, cross1, diag1
        xrow0_t = sb.tile([1, 2304], BF16)
        xrow1_t = sb.tile([1, 2304], BF16)
        xrow_tiles = [xrow0_t, xrow1_t]
        trash = sb.tile([128, 2304], BF16)
        trashf = sb.tile([128, 1216], F32)

        # colsum psum: [128, 512]: col = b*256 + t*16 + bi (diag at bi==t)
        cs = ps_m.tile([128, 512], F32, tag="cs", bufs=1)
        nc.vector.memset(cs, 0.0)

        # ---- input DMAs ----
        # img0 first on sync; desc on gpsimd (pool idle early)
        img_tiles, sd_tiles, dsl_tiles, dnm_tiles = [], [], [], []
        img0 = sbi.tile([128, 3, 3, 384], BF16, tag="img", bufs=2)
        nc.sync.dma_start(out=img0,
                          in_=imgs_d[0].rearrange("c (hc p) w -> p c hc w", p=128))
        nc.sync.dma_start(out=b1sp_t,
                          in_=b1sp_d[:, :].rearrange("(k p) h -> p k h", p=128))
        nc.sync.dma_start(out=b1dp_t,
                          in_=b1dp_d[:, :].rearrange("(k p) h -> p k h", p=128))
        nc.sync.dma_start(out=msm_t, in_=msm_d[:, :].rearrange("(k p) h -> p k h", p=128))
        nc.sync.dma_start(out=mdf_t, in_=mdf_d[:, :].rearrange("(k p) h -> p k h", p=128))
        nc.sync.dma_start(out=mgap_t, in_=mgap_d[:, :].rearrange("(k p) h -> p k h", p=128))
        nc.sync.dma_start(out=mga_t, in_=mga_d[:, :].rearrange("(k p) h -> p k h", p=128))
        for b in range(NIMG):
            dnm = sbi.tile([128, 16, DDIM], FP8, tag="dnm", bufs=2)
            nc.gpsimd.dma_start(out=dnm, in_=dnm_d[b])
            dnm_tiles.append(dnm)
            dsl = sbi.tile([128, 2, NDESC], FP8, tag="dsl", bufs=2)
            nc.gpsimd.dma_start(out=dsl, in_=dsl_d[b])
            dsl_tiles.append(dsl)
        for b in range(NIMG):
            sdt = sbi.tile([128, 3, 384], BF16, tag="sdt", bufs=2)
            nc.gpsimd.dma_start(out=sdt,
                                in_=sd_d[b].rearrange("(c p) w -> p c w", p=128))
            sd_tiles.append(sdt)
        img1 = sbi.tile([128, 3, 3, 384], BF16, tag="img", bufs=2)
        nc.sync.dma_start(out=img1,
                          in_=imgs_d[1].rearrange("c (hc p) w -> p c hc w", p=128))
        img_tiles.extend([img0, img1])
        for t, d in [(s8_t, s8_d), (t16_t, t16_d), (id2_t, id2_d), (ninfh_t, ninfh_d),
                     (lw0_t, lw0_d), (e2b_t, e2b_d), (e64_t, e64_d),
                     (iota128_t, iota128_d), (thrW0_t, thrW0_d), (thrW1_t, thrW1_d),
                     (ones128_t, ones128_d), (ones1_t, ones1_d),
                     (ones64h_t, ones64h_d)]:
            nc.sync.dma_start(out=t, in_=d[:, :])

        # ---- r = rsqrt(nsq) from n-major fp8: wide square + wide reduce ----
        r_tiles = []
        dsq = sb.tile([128, 16, DDIM], BF16)

        def emit_r(b):
            dnm = dnm_tiles[b]
            nc.scalar.activation(dsq, dnm, AF.Square)
            nsqf = sbi.tile([128, 16], F32, tag="nsqf", bufs=2)
            nc.vector.tensor_reduce(nsqf, dsq, axis=AX.X, op=ALU.add)
            sr = sbi.tile([128, 16], F32, tag="sr", bufs=2)
            nc.scalar.activation(sr, nsqf, AF.Sqrt)
            y0 = sbi.tile([128, 16], F32, tag="y0", bufs=2)
            nc.vector.reciprocal(y0, sr)
            yy = sbi.tile([128, 16], F32, tag="yy", bufs=2)
            nc.vector.tensor_tensor(out=yy, in0=y0, in1=y0, op=ALU.mult)
            nc.vector.tensor_tensor(out=yy, in0=yy, in1=nsqf, op=ALU.mult)
            nc.vector.tensor_scalar(yy, yy, -0.5, 1.5, op0=ALU.mult, op1=ALU.add)
            r_bf = sbi.tile([128, 16], BF16, tag="rbf", bufs=2)
            nc.vector.tensor_tensor(out=r_bf, in0=yy, in1=y0, op=ALU.mult)
            r_tiles.append(r_bf)

        # ---- gram tile generator ----
        def gram_tiles_gen():
            for b in range(NIMG):
                for bi in range(16):
                    c0 = 128 * bi
                    pos = c0
                    while pos < NDESC:
                        wdt = min(1024, NDESC - pos)
                        yield (b, bi, pos, wdt)
                        pos += wdt

        _gram_iter = gram_tiles_gen()
        _gram_state = {"done": False, "acc": 0.0, "pending": []}

        def _emit_matvecs(ent):
            grelu, b, bi, pos, wdt = ent
            r_bf = r_tiles[b]
            for ci in range(wdt // 128):
                t = (pos + ci * 128) // 128
                col = 256 * b + 16 * t + bi
                nc.tensor.matmul(cs[:, col:col + 1],
                                 grelu[:, 128 * ci:128 * (ci + 1)],
                                 r_bf[:, bi:bi + 1],
                                 start=True, stop=True)

        def pump_gram(n, act_share=0.0):
            for _ in range(n):
                if _gram_state["done"]:
                    break
                try:
                    b, bi, pos, wdt = next(_gram_iter)
                except StopIteration:
                    _gram_state["done"] = True
                    break
                dsl = dsl_tiles[b]
                gp = ps_g.tile([128, 1024], F32, tag="g")
                off = 0
                while off < wdt:
                    nn = min(512, wdt - off)
                    nc.tensor.matmul(gp[:, off:off + nn],
                                     dsl[:, :, 128 * bi:128 * (bi + 1)],
                                     dsl[:, :, pos + off:pos + off + nn],
                                     start=True, stop=True, perf_mode=DR)
                    off += nn
                grelu = sbi.tile([128, 1024], BF16, tag="grelu", bufs=5)
                _gram_state["acc"] += act_share
                if _gram_state["acc"] >= 1.0:
                    _gram_state["acc"] -= 1.0
                    nc.scalar.activation(grelu[:, 0:wdt], gp[:, 0:wdt], AF.Relu)
                else:
                    nc.vector.tensor_scalar(grelu[:, 0:wdt], gp[:, 0:wdt], 0.0,
                                            None, op0=ALU.max)
                _gram_state["pending"].append((grelu, b, bi, pos, wdt))
                while len(_gram_state["pending"]) > 3:
                    _emit_matvecs(_gram_state["pending"].pop(0))
            if _gram_state["done"]:
                while _gram_state["pending"]:
                    _emit_matvecs(_gram_state["pending"].pop(0))

        # ---- conv + NMS per image ----
        resp_list, nms_list, bw_list = [], [], []

        def emit_conv(b, pump=True):
            img_t = img_tiles[b]
            imgv = img_t.rearrange("p c hc w -> p (c hc) w")

            # P1: smooth/diff along H -> [w-part, wb, h]
            sT = sbi.tile([128, 3, 384], BF16, tag="sT")
            dT = sbi.tile([128, 3, 384], BF16, tag="dT")
            for di, (dst, bnd, wins) in enumerate(
                    ((sT, b1sp_t, w1s), (dT, b1dp_t, w1d))):
                for wb in range(3):
                    pst = ps_cv.tile([128, 384], F32, tag="cv")
                    first = True
                    for k in range(9):
                        if wins[k] is None:
                            continue
                        c0, c1 = wins[k]
                        nc.tensor.matmul(pst[:, c0:c1],
                                         imgv[:, k, wb * 128:(wb + 1) * 128],
                                         bnd[:, k, 0:c1 - c0], start=first, stop=False)
                        first = False
                    if (di * 3 + wb) % 2 == 0:
                        nc.scalar.copy(dst[:, wb, :], pst)
                    else:
                        nc.vector.tensor_copy(dst[:, wb, :], pst)

            # P2: diff/smooth along W -> px=dx, py=dy [w-part(ob), h]
            pq = sbi.tile([128, 3, 384], BF16, tag="pq", bufs=1)
            qq = sbi.tile([128, 3, 384], BF16, tag="qq", bufs=1)
            rr = sbi.tile([128, 3, 384], BF16, tag="rr", bufs=1)
            dxs = sbi.tile([128, 384], BF16, tag="dxs", bufs=2)
            for ob in range(3):
                px = ps_cv.tile([128, 384], F32, tag="cv")
                fx = True
                for kc, ob2 in pr_df:
                    if ob2 != ob:
                        continue
                    nc.tensor.matmul(px, mdf_t[:, kc, ob * 128:(ob + 1) * 128],
                                     sT[:, kc, :], start=fx, stop=False)
                    fx = False
                nc.scalar.activation(pq[:, ob, :], px, AF.Square)
                nc.scalar.copy(dxs, px)
                py = ps_cv.tile([128, 384], F32, tag="cv")
                fy = True
                for kc, ob2 in pr_sm:
                    if ob2 != ob:
                        continue
                    nc.tensor.matmul(py, msm_t[:, kc, ob * 128:(ob + 1) * 128],
                                     dT[:, kc, :], start=fy, stop=False)
                    fy = False
                nc.scalar.activation(qq[:, ob, :], py, AF.Square)
                nc.vector.tensor_tensor(out=rr[:, ob, :], in0=dxs, in1=py, op=ALU.mult)
            # plus/minus (linear through gauss)
            pl = sbi.tile([128, 3, 384], BF16, tag="pl", bufs=1)
            mi = sbi.tile([128, 3, 384], BF16, tag="mi", bufs=1)
            nc.vector.tensor_tensor(out=pl, in0=pq, in1=qq, op=ALU.add)
            nc.vector.tensor_tensor(out=mi, in0=pq, in1=qq, op=ALU.subtract)

            if pump:
                pump_gram(2, act_share=0.2)

            # P3: gauss along W -> [h-part(hb), w]
            gP = sbi.tile([128, 3, 384], BF16, tag="gP", bufs=1)
            gM = sbi.tile([128, 3, 384], BF16, tag="gM", bufs=1)
            gR = sbi.tile([128, 3, 384], BF16, tag="gR", bufs=1)
            for si, (src, dst) in enumerate(((pl, gP), (mi, gM), (rr, gR))):
                for hb in range(3):
                    pst = ps_cv.tile([128, 384], F32, tag="cv")
                    for i, cw in enumerate(range(3)):
                        c0, c1 = wga3[cw]
                        nc.tensor.matmul(pst[:, c0:c1],
                                         src[:, cw, hb * 128:(hb + 1) * 128],
                                         mgap_t[:, cw, 0:c1 - c0],
                                         start=(i == 0), stop=False)
                    if (si * 3 + hb) % 3 == 0:
                        nc.vector.tensor_copy(dst[:, hb, :], pst)
                    else:
                        nc.scalar.copy(dst[:, hb, :], pst)

            if pump:
                pump_gram(2, act_share=0.2)

            # P4: gauss along H -> P (tr), M, R in [h-part(ob), w]; response
            resp = sbi.tile([128, 3, 388], BF16, tag="resp", bufs=2)
            for ob in range(3):
                pR = ps_cv.tile([128, 384], F32, tag="cv")
                first = True
                for kc, ob2 in pr_ga:
                    if ob2 != ob:
                        continue
                    nc.tensor.matmul(pR, mga_t[:, kc, ob * 128:(ob + 1) * 128],
                                     gR[:, kc, :], start=first, stop=False)
                    first = False
                B4 = sbi.tile([128, 384], BF16, tag="B4", bufs=2)
                nc.scalar.activation(B4, pR, AF.Square, scale=2.0)
                pM = ps_cv.tile([128, 384], F32, tag="cv")
                first = True
                for kc, ob2 in pr_ga:
                    if ob2 != ob:
                        continue
                    nc.tensor.matmul(pM, mga_t[:, kc, ob * 128:(ob + 1) * 128],
                                     gM[:, kc, :], start=first, stop=False)
                    first = False
                A2 = sbi.tile([128, 384], BF16, tag="A2", bufs=2)
                nc.scalar.activation(A2, pM, AF.Square)
                disc = sbi.tile([128, 384], BF16, tag="disc", bufs=2)
                nc.vector.tensor_tensor(out=disc, in0=A2, in1=B4, op=ALU.add)
                s2 = sbi.tile([128, 384], F32, tag="s2", bufs=2)
                nc.scalar.activation(s2, disc, AF.Sqrt, scale=0.25)
                pP = ps_cv.tile([128, 384], F32, tag="cv")
                first = True
                for kc, ob2 in pr_ga:
                    if ob2 != ob:
                        continue
                    nc.tensor.matmul(pP, mga_t[:, kc, ob * 128:(ob + 1) * 128],
                                     gP[:, kc, :], start=first, stop=False)
                    first = False
                nc.vector.scalar_tensor_tensor(out=resp[:, ob, 2:386], in0=pP,
                                               scalar=0.5, in1=s2,
                                               op0=ALU.mult, op1=ALU.subtract)
                nc.vector.tensor_copy(resp[:, ob, 0:2], ninfh_t[:, 0:2])
                nc.vector.tensor_copy(resp[:, ob, 386:388], ninfh_t[:, 0:2])
            resp_list.append(resp)

        def emit_nms_gen(b):
            TTs = nc.vector.tensor_tensor
            resp = resp_list[b]
            sdt = sd_tiles[b]
            sdv = sdt.rearrange("p c w -> p (c w)")
            spA = sbi.tile([128, 1152], F32, tag="spA", bufs=1)
            nc.scalar.activation(spA, sdv, AF.Exp)
            nc.scalar.activation(trashf[:, 0:1152], spA, AF.Ln, bias=1.0,
                                 accum_out=spacc[:, b:b + 1])

            t1 = sbi.tile([128, 3, 388], BF16, tag="t1", bufs=1)
            TTs(out=t1[:, :, 0:387], in0=resp[:, :, 0:387],
                in1=resp[:, :, 1:388], op=ALU.max)
            t2 = sbi.tile([128, 3, 388], BF16, tag="t2", bufs=1)
            TTs(out=t2[:, :, 0:385], in0=t1[:, :, 0:385],
                in1=t1[:, :, 2:387], op=ALU.max)
            m1 = sbi.tile([128, 3, 384], BF16, tag="m1", bufs=2)
            TTs(out=m1, in0=t2[:, :, 0:384], in1=resp[:, :, 4:388], op=ALU.max)
            yield
            shs = []
            for k in (1, 2):
                sh = sbi.tile([128, 3, 384], BF16, tag="shd", bufs=2)
                nc.vector.memset(sh[0:k, 0, :], NEG)
                nc.sync.dma_start(out=sh[k:128], in_=m1[0:128 - k])
                nc.sync.dma_start(out=sh[0:k, 1:3, :], in_=m1[128 - k:128, 0:2, :])
                shs.append(sh)
                sh2 = sbi.tile([128, 3, 384], BF16, tag="shu", bufs=2)
                nc.gpsimd.dma_start(out=sh2[128 - k:128, 2, :],
                                    in_=ninfh_t[0:k, 0:384])
                nc.gpsimd.dma_start(out=sh2[0:128 - k], in_=m1[k:128])
                nc.gpsimd.dma_start(out=sh2[128 - k:128, 0:2, :], in_=m1[0:k, 1:3, :])
                shs.append(sh2)
            yield
            mp1 = sbi.tile([128, 3, 384], BF16, tag="mp1", bufs=1)
            nc.vector.tensor_tensor(out=mp1, in0=m1, in1=shs[0], op=ALU.max)
            mp2 = sbi.tile([128, 3, 384], BF16, tag="mp2", bufs=1)
            TTs(out=mp2, in0=shs[1], in1=shs[2], op=ALU.max)
            nc.vector.tensor_tensor(out=mp2, in0=mp2, in1=shs[3], op=ALU.max)
            mp = sbi.tile([128, 3, 384], BF16, tag="mp", bufs=2)
            nc.vector.tensor_tensor(out=mp, in0=mp1, in1=mp2, op=ALU.max)
            yield
            e1 = sbi.tile([128, 3, 384], BF16, tag="e1", bufs=1)
            TTs(out=e1, in0=resp[:, :, 2:386], in1=mp, op=ALU.is_ge)
            nms = sbi.tile([128, 3, 384], BF16, tag="nms", bufs=2)
            nc.vector.tensor_tensor(out=nms, in0=resp[:, :, 2:386], in1=e1, op=ALU.mult)
            nms_list.append(nms)
            yield
            bw = sbi.tile([128, 3, 48], BF16, tag="bw")
            nc.vector.tensor_reduce(bw, nms.rearrange("p c (g k) -> p c g k", k=8),
                                    axis=AX.X, op=ALU.max)
            shbs = []
            for k in range(1, 8):
                shk = sbi.tile([128, 3, 48], BF16, tag="shb", bufs=8)
                q = nc.sync if k % 2 else nc.gpsimd
                q.dma_start(out=shk[0:128 - k], in_=bw[k:128])
                q.dma_start(out=shk[128 - k:128], in_=bw[128 - k:128])
                shbs.append(shk)
            yield
            q1 = sbi.tile([128, 3, 48], BF16, tag="bwm", bufs=2)
            nc.vector.tensor_tensor(out=q1, in0=bw, in1=shbs[0], op=ALU.max)
            q2 = sbi.tile([128, 3, 48], BF16, tag="bwm", bufs=2)
            nc.vector.tensor_tensor(out=q2, in0=shbs[1], in1=shbs[2], op=ALU.max)
            q3 = sbi.tile([128, 3, 48], BF16, tag="bwm3", bufs=2)
            nc.vector.tensor_tensor(out=q3, in0=shbs[3], in1=shbs[4], op=ALU.max)
            q4 = sbi.tile([128, 3, 48], BF16, tag="bwm3", bufs=2)
            nc.vector.tensor_tensor(out=q4, in0=shbs[5], in1=shbs[6], op=ALU.max)
            q5 = sbi.tile([128, 3, 48], BF16, tag="bwm5", bufs=2)
            nc.vector.tensor_tensor(out=q5, in0=q1, in1=q2, op=ALU.max)
            q6 = sbi.tile([128, 3, 48], BF16, tag="bwm5", bufs=2)
            nc.vector.tensor_tensor(out=q6, in0=q3, in1=q4, op=ALU.max)
            yield
            cur = sbi.tile([128, 3, 48], BF16, tag="bwf", bufs=2)
            nc.vector.tensor_tensor(out=cur, in0=q5, in1=q6, op=ALU.max)
            yield
            p16 = ps_m.tile([16, 144], F32, tag="m", bufs=1)
            nc.tensor.matmul(p16, s8_t, cur.rearrange("p c g -> p (c g)"),
                             start=True, stop=True)
            p16s = sbi.tile([16, 3, 48], BF16, tag="p16s", bufs=2)
            nc.scalar.copy(p16s.rearrange("p c g -> p (c g)"), p16)
            bw_list.append(p16s)
            xr = xrow_tiles[b]
            p16f = p16s.rearrange("p c g -> p (c g)")
            nc.sync.dma_start(out=xr[0:1, 0:1152], in_=p16f[0:8, :])
            nc.gpsimd.dma_start(out=xr[0:1, 1152:2304], in_=p16f[8:16, :])

        # ---- per-image threshold search + selection (interleavable) ----
        W0 = 0.25 / 64.0
        W1 = 0.25 / 4096.0
        maskz = sb.tile([128, 2, 2], F32)   # [*, img, round] zero-padded masks
        nc.vector.memset(maskz, 0.0)
        trash64 = sb.tile([64, 2304], BF16)

        def thresh_sel_gen_img(b):
            xr = xrow_tiles[b]
            x64 = sbi.tile([64, 2304], BF16, tag="x64", bufs=2)
            nc.gpsimd.partition_broadcast(x64, xr, channels=64)
            yield
            # round 1 (constant thresholds)
            cnt = sbi.tile([64, 1], F32, tag="cnt", bufs=2)
            nc.vector.tensor_scalar(trash64, x64, thrW0_t[:, 0:1], None,
                                    op0=ALU.is_gt, op1=ALU.add, accum_out=cnt)
            nc.vector.tensor_scalar(maskz[0:64, b, 0:1], cnt, float(NUM) - 0.5,
                                    None, op0=ALU.is_ge)
            yield
            kp1 = ps_m.tile([1, 1], F32, tag="m", bufs=1)
            nc.tensor.matmul(kp1, maskz[:, b, 0:1], ones128_t, start=True, stop=True)
            lo_sc = sbi.tile([1, 1], F32, tag="losc", bufs=2)
            nc.vector.tensor_scalar(lo_sc, kp1, W0, -W0, op0=ALU.mult, op1=ALU.add)
            lo64p = ps_m.tile([64, 1], F32, tag="m", bufs=1)
            nc.tensor.matmul(lo64p, ones1[0:1, 0:64] if False else ones1_t[0:1, 0:64],
                             lo_sc, start=True, stop=True)
            T2 = sbi.tile([64, 1], F32, tag="T2", bufs=2)
            nc.vector.tensor_tensor(out=T2, in0=lo64p, in1=thrW1_t, op=ALU.add)
            yield
            # round 2
            cnt2 = sbi.tile([64, 1], F32, tag="cnt2", bufs=2)
            nc.vector.tensor_scalar(trash64, x64, T2[:, 0:1], None,
                                    op0=ALU.is_gt, op1=ALU.add, accum_out=cnt2)
            nc.vector.tensor_scalar(maskz[0:64, b, 1:2], cnt2, float(NUM) - 0.5,
                                    None, op0=ALU.is_ge)
            yield
            kp2 = ps_m.tile([1, 1], F32, tag="m", bufs=1)
            nc.tensor.matmul(kp2, maskz[:, b, 1:2], ones128_t, start=True, stop=True)
            tf = sbi.tile([1, 1], F32, tag="tf", bufs=2)
            nc.vector.tensor_scalar(tf, kp2, W1, -W1, op0=ALU.mult, op1=ALU.add)
            nc.vector.tensor_tensor(out=tf, in0=tf, in1=lo_sc, op=ALU.add)
            nc.vector.tensor_scalar(tf, tf, 1e-30, None, op0=ALU.max)
            tb16p = ps_m.tile([16, 1], F32, tag="m", bufs=1)
            nc.tensor.matmul(tb16p, ones1_t[0:1, 0:16], tf, start=True, stop=True)
            tb16 = sbi.tile([16, 1], F32, tag="tb16", bufs=2)
            nc.scalar.copy(tb16, tb16p)
            yield
            # selection + dot
            p16s = bw_list[b]
            nms = nms_list[b]
            sdt = sd_tiles[b]
            p16c = sbi.tile([16, 3, 48], BF16, tag="p16c", bufs=2)
            nc.vector.tensor_scalar(p16c.rearrange("p c g -> p (c g)"),
                                    p16s.rearrange("p c g -> p (c g)"),
                                    tb16[:, 0:1], None, op0=ALU.max)
            yield
            for cc in range(3):
                bexp = ps_m.tile([128, 384], F32, tag="m", bufs=1)
                nc.tensor.matmul(bexp, t16_t,
                                 p16c[:, cc, :].unsqueeze(2)
                                 .to_broadcast([16, 48, 8]),
                                 start=True, stop=True)
                sel = sbi.tile([128, 384], BF16, tag="sel", bufs=2)
                nc.vector.tensor_tensor(out=sel, in0=nms[:, cc, :], in1=bexp,
                                        op=ALU.is_ge)
                dtmp = sbi.tile([128, 384], BF16, tag="dtmp", bufs=2)
                nc.vector.tensor_tensor(out=dtmp, in0=sel, in1=sdt[:, cc, :],
                                        op=ALU.mult)
                nc.vector.tensor_reduce(dacc[:, 3 * b + cc:3 * b + cc + 1],
                                        dtmp, axis=AX.X, op=ALU.add)
                yield

        # ================= schedule =================
        def drive(gen, tiles_per_step=1, act_share=0.45):
            for _ in gen:
                pump_gram(tiles_per_step, act_share=act_share)

        emit_conv(0, pump=False)
        pump_gram(3)
        emit_r(0)
        emit_r(1)
        drive(emit_nms_gen(0), 1, 0.3)
        emit_conv(1)
        drive(thresh_sel_gen_img(0), 0, 0.4)
        drive(emit_nms_gen(1), 2, 0.6)
        drive(thresh_sel_gen_img(1), 2, 0.6)
        pump_gram(1000, act_share=0.6)

        # gram weighted sums per image: full (incl diag) and diag-only
        for b in range(NIMG):
            wcs = sbi.tile([128, 256], F32, tag="wcs", bufs=2)
            nc.vector.tensor_tensor(
                out=wcs.rearrange("p (a k) -> p a k", k=16),
                in0=cs[:, 256 * b:256 * b + 256].rearrange("p (a k) -> p a k", k=16),
                in1=r_tiles[b].unsqueeze(2).to_broadcast([128, 16, 16]),
                op=ALU.mult)
            nc.vector.tensor_reduce(gall[:, 2 * b:2 * b + 1], wcs,
                                    axis=AX.X, op=ALU.add)
            wcd = sbi.tile([128, 16], F32, tag="wcd", bufs=2)
            import concourse.ap as ap_mod
            csap = cs[:, 256 * b:256 * b + 256]
            diag_ap = ap_mod.AP(csap.tensor, csap.offset,
                                [list(csap.ap[0]), [17 * csap.ap[1][0], 16]])
            nc.vector.tensor_tensor(out=wcd, in0=diag_ap, in1=r_tiles[b],
                                    op=ALU.mult)
            nc.vector.tensor_reduce(gall[:, 2 * b + 1:2 * b + 2], wcd,
                                    axis=AX.X, op=ALU.add)

        # ---- final reduce ----
        vals = sb.tile([128, 4], F32)
        nc.vector.tensor_reduce(vals[:, 0:1], spacc, axis=AX.X, op=ALU.add)
        nc.vector.tensor_reduce(vals[:, 1:2], dacc, axis=AX.X, op=ALU.add)
        nc.vector.tensor_tensor(out=vals[:, 2:3], in0=gall[:, 0:1], in1=gall[:, 2:3],
                                op=ALU.add)
        nc.vector.tensor_tensor(out=vals[:, 3:4], in0=gall[:, 1:2], in1=gall[:, 3:4],
                                op=ALU.add)
        fps = ps_m.tile([4, 1], F32, tag="m", bufs=1)
        nc.tensor.matmul(fps, vals, ones128_t, start=True, stop=True)
        fsb = sb.tile([4, 1], F32)
        nc.scalar.copy(fsb, fps)
        nc.sync.dma_start(out=out_d[:, :], in_=fsb)

        ps_m.release()
        ps_g.release()
        ps_cv.release()
        sbi.release()
        sb.release()

    nc.finalize()
    return nc, C


_CACHE = {}


def kernel(descriptors, scores, scores_dense, imgs):
    B = descriptors.shape[0]
    ncore = 8
    per = B // ncore
    if "nc" not in _CACHE:
        _CACHE["nc"], _CACHE["C"] = build_program()
    nc, C = _CACHE["nc"], _CACHE["C"]

    imgs_bf = np.ascontiguousarray(np.asarray(imgs).astype(ml_dtypes.bfloat16))
    sd = np.ascontiguousarray(np.asarray(scores_dense).reshape(B, H, W)
                              .astype(ml_dtypes.bfloat16))
    desc8 = np.asarray(descriptors).astype(ml_dtypes.float8_e4m3)
    # slab d-major: [B, 128(dj), 2(slab), N]
    dsl = np.ascontiguousarray(desc8.transpose(0, 2, 1)
                               .reshape(B, 2, 128, NDESC).transpose(0, 2, 1, 3))
    # n-major: [B, 128(n in chunk), 16(chunk), D]
    dnm = np.ascontiguousarray(desc8.reshape(B, 16, 128, DDIM).transpose(0, 2, 1, 3))

    in_maps = []
    for c in range(ncore):
        s = slice(c * per, (c + 1) * per)
        in_maps.append({
            "imgs": imgs_bf[s], "sd": sd[s], "dsl": dsl[s], "dnm": dnm[s],
            "b1sp": C["b1sp"], "b1dp": C["b1dp"], "mgap": C["mgap"],
            "msm": C["msm"], "mdf": C["mdf"], "mga": C["mga"],
            "s8": C["s8"], "t16": C["t16"], "id2": C["id2"], "ninfh": C["ninfh"],
            "lw0": C["lw0"], "e2b": C["e2b"], "e64": C["e64"],
            "iota128": C["iota128"], "thrW0": C["thrW0"], "thrW1": C["thrW1"],
            "ones128": C["ones128"], "ones1": C["ones1"],
            "ones64h": C["ones64h"],
        })

    res = run_bass_kernel_spmd(nc, in_maps, core_ids=list(range(ncore)))
    S1 = S2 = Sall = Sdia = 0.0
    for c in range(ncore):
        o = np.asarray(res.results[c]["out"])[:, 0].astype(np.float64)
        S1 += o[0]
        S2 += o[1]
        Sall += o[2]
        Sdia += o[3]
    bce = (S1 - S2) / (B * H * W)
    relu_mean = (2.0 * Sall - Sdia) / (B * NDESC * NDESC)
    return np.array(bce + relu_mean, dtype=np.float32)


# revision 6
# speedup vs baseline: 1.1424x; 1.0294x over previous
"""Trainium2 Bass kernel for nn_DistinctionLoss (GFTT corners BCE + relu-cosine mean).

v2: batch-sharded 2 images/core across 8 cores.
 - fp8 DoubleRow raw gram (host-side e4m3 cast + d-major slab layout);
   normalization folded in post-relu via N=1 PE matvecs with r = rsqrt(diag).
 - GFTT restructured: (dx^2+dy^2, dx^2-dy^2) pushed through the linear gaussian
   convs, deleting the tr/A add/sub stages.
 - Elementwise spread across Act/DVE/Pool; bf16 everywhere DVE gets 2x mode.
"""
import os
import numpy as np
import ml_dtypes

import concourse.bacc as bacc
import concourse.mybir as mybir
from concourse.tile import TileContext
from concourse.bass_utils import run_bass_kernel_spmd

F32 = mybir.dt.float32
BF16 = mybir.dt.bfloat16
FP8 = mybir.dt.float8e4
AF = mybir.ActivationFunctionType
ALU = mybir.AluOpType
AX = mybir.AxisListType
DR = mybir.MatmulPerfMode.DoubleRow

H = W = 384
NIMG = 2
NDESC = 2048
DDIM = 256
NUM = 200
NEG = -1e30
BW = 136  # packed band window width

_bf = lambda a: np.ascontiguousarray(a.astype(ml_dtypes.bfloat16))


def _band(k, mode, n=384):
    pad = len(k) // 2
    idx = np.arange(n + 2 * pad) - pad
    if mode == "edge":
        src = np.clip(idx, 0, n - 1)
    else:  # reflect
        src = np.abs(idx)
        src = np.where(src >= n, 2 * (n - 1) - src, src)
    M = np.zeros((n, n), np.float32)
    for i, kv in enumerate(k):
        M[src[np.arange(n) + i], np.arange(n)] += kv
    return M


def _gauss7():
    xs = np.arange(7, dtype=np.float32) - 3.0
    g = np.exp(-0.5 * xs ** 2)
    return (g / g.sum()).astype(np.float32)


def _wins(M, nchunk):
    wins = []
    for k in range(nchunk):
        rows = M[k * 128:(k + 1) * 128]
        nz = np.nonzero(np.any(rows != 0, axis=0))[0]
        wins.append((int(nz[0]), int(nz[-1]) + 1) if len(nz) else None)
    return wins


def _pack(M, nchunk, wins):
    """Pack band matrix rows into [nchunk*128, BW] windows."""
    P = np.zeros((nchunk * 128, BW), np.float32)
    for k in range(nchunk):
        if wins[k] is None:
            continue
        c0, c1 = wins[k]
        P[k * 128:(k + 1) * 128, 0:c1 - c0] = M[k * 128:(k + 1) * 128, c0:c1]
    return P


def _nzpairs(M):
    out = []
    for ob in range(3):
        for kc in range(3):
            if np.any(M[kc * 128:(kc + 1) * 128, ob * 128:(ob + 1) * 128]):
                out.append((kc, ob))
    return out


def _consts():
    c = {}
    Msm = _band(np.array([1, 2, 1], np.float32) / 8.0, "edge")
    Mdf = _band(np.array([-1, 0, 1], np.float32), "edge")
    Mga = _band(_gauss7(), "reflect")
    coef = np.array([0.299, 0.587, 0.114], np.float32)
    b1s = np.concatenate([coef[i] * Msm for i in range(3)], axis=0)
    b1d = np.concatenate([coef[i] * Mdf for i in range(3)], axis=0)
    c["w1s"] = _wins(b1s, 9)
    c["w1d"] = _wins(b1d, 9)
    c["wga3"] = _wins(Mga, 3)
    c["pr_df"] = _nzpairs(Mdf)
    c["pr_sm"] = _nzpairs(Msm)
    c["pr_ga"] = _nzpairs(Mga)
    c["b1sp"] = _bf(_pack(b1s, 9, c["w1s"]))
    c["b1dp"] = _bf(_pack(b1d, 9, c["w1d"]))
    c["mgap"] = _bf(_pack(Mga, 3, c["wga3"]))
    c["msm"] = _bf(Msm)
    c["mdf"] = _bf(Mdf)
    c["mga"] = _bf(Mga)
    c["mgan"] = _bf(-Mga)
    S8 = np.zeros((128, 16), np.float32)
    S8[np.arange(16) * 8, np.arange(16)] = 1.0
    c["s8"] = _bf(S8)
    T16 = np.zeros((16, 128), np.float32)
    T16[np.arange(128) // 8, np.arange(128)] = 1.0
    c["t16"] = _bf(T16)
    c["id2"] = np.eye(2, dtype=np.float32)
    c["ninfh"] = _bf(np.full((128, 384), NEG, np.float32))
    c["lw0"] = np.array([[0.0, 0.25 / 64.0], [0.0, 0.25 / 64.0]], np.float32)
    halves = np.zeros((2, 128), np.float32)
    halves[0, :64] = 1.0
    halves[1, 64:] = 1.0
    c["e2b"] = halves.copy()
    c["e64"] = np.ascontiguousarray(halves.T)
    c["iota128"] = (np.arange(128, dtype=np.float32) % 64).reshape(128, 1)
    c["thrW0"] = (np.arange(64, dtype=np.float32) * (0.25 / 64.0)).reshape(64, 1)
    c["thrW1"] = (np.arange(64, dtype=np.float32) * (0.25 / 4096.0)).reshape(64, 1)
    c["ones128"] = np.ones((128, 1), np.float32)
    c["ones64h"] = _bf(np.ones((1, 64), np.float32))
    c["ones1"] = np.ones((1, 128), np.float32)
    return c


def build_program():
    C = _consts()
    nc = bacc.Bacc()

    imgs_d = nc.dram_tensor("imgs", [NIMG, 3, H, W], BF16, kind="ExternalInput")
    sd_d = nc.dram_tensor("sd", [NIMG, H, W], BF16, kind="ExternalInput")
    dsl_d = nc.dram_tensor("dsl", [NIMG, 128, 2, NDESC], FP8, kind="ExternalInput")
    dnm_d = nc.dram_tensor("dnm", [NIMG, 128, 16, DDIM], FP8, kind="ExternalInput")
    b1sp_d = nc.dram_tensor("b1sp", [1152, BW], BF16, kind="ExternalInput")
    b1dp_d = nc.dram_tensor("b1dp", [1152, BW], BF16, kind="ExternalInput")
    mgap_d = nc.dram_tensor("mgap", [384, BW], BF16, kind="ExternalInput")
    msm_d = nc.dram_tensor("msm", [384, 384], BF16, kind="ExternalInput")
    mdf_d = nc.dram_tensor("mdf", [384, 384], BF16, kind="ExternalInput")
    mga_d = nc.dram_tensor("mga", [384, 384], BF16, kind="ExternalInput")
    mgan_d = nc.dram_tensor("mgan", [384, 384], BF16, kind="ExternalInput")
    s8_d = nc.dram_tensor("s8", [128, 16], BF16, kind="ExternalInput")
    t16_d = nc.dram_tensor("t16", [16, 128], BF16, kind="ExternalInput")
    id2_d = nc.dram_tensor("id2", [2, 2], F32, kind="ExternalInput")
    ninfh_d = nc.dram_tensor("ninfh", [128, 384], BF16, kind="ExternalInput")
    lw0_d = nc.dram_tensor("lw0", [2, 2], F32, kind="ExternalInput")
    e2b_d = nc.dram_tensor("e2b", [2, 128], F32, kind="ExternalInput")
    e64_d = nc.dram_tensor("e64", [128, 2], F32, kind="ExternalInput")
    iota128_d = nc.dram_tensor("iota128", [128, 1], F32, kind="ExternalInput")
    thrW0_d = nc.dram_tensor("thrW0", [64, 1], F32, kind="ExternalInput")
    thrW1_d = nc.dram_tensor("thrW1", [64, 1], F32, kind="ExternalInput")
    ones128_d = nc.dram_tensor("ones128", [128, 1], F32, kind="ExternalInput")
    ones1_d = nc.dram_tensor("ones1", [1, 128], F32, kind="ExternalInput")
    ones64h_d = nc.dram_tensor("ones64h", [1, 64], BF16, kind="ExternalInput")
    out_d = nc.dram_tensor("out", [4, 1], F32, kind="ExternalOutput")

    w1s, w1d, wga3 = C["w1s"], C["w1d"], C["wga3"]
    pr_df, pr_sm, pr_ga = C["pr_df"], C["pr_sm"], C["pr_ga"]

    with TileContext(nc) as tc:
        sb = tc.alloc_tile_pool(name="sb", bufs=1)
        sbi = tc.alloc_tile_pool(name="sbi", bufs=2)
        ps_cv = tc.alloc_tile_pool(name="pscv", bufs=2, space="PSUM")
        ps_g = tc.alloc_tile_pool(name="psg", bufs=2, space="PSUM")
        ps_m = tc.alloc_tile_pool(name="psm", bufs=2, space="PSUM")

        # ---- persistent SBUF ----
        b1sp_t = sb.tile([128, 9, BW], BF16)
        b1dp_t = sb.tile([128, 9, BW], BF16)
        mgap_t = sb.tile([128, 3, BW], BF16)
        msm_t = sb.tile([128, 3, 384], BF16)
        mdf_t = sb.tile([128, 3, 384], BF16)
        mga_t = sb.tile([128, 3, 384], BF16)
        mgan_t = sb.tile([128, 3, 384], BF16)
        s8_t = sb.tile([128, 16], BF16)
        t16_t = sb.tile([16, 128], BF16)
        id2_t = sb.tile([2, 2], F32)
        ninfh_t = sb.tile([128, 384], BF16)
        lw0_t = sb.tile([2, 2], F32)
        e2b_t = sb.tile([2, 128], F32)
        e64_t = sb.tile([128, 2], F32)
        iota128_t = sb.tile([128, 1], F32)
        thrW0_t = sb.tile([64, 1], F32)
        thrW1_t = sb.tile([64, 1], F32)
        ones128_t = sb.tile([128, 1], F32)
        ones1_t = sb.tile([1, 128], F32)
        ones64h_t = sb.tile([1, 64], BF16)

        spacc = sb.tile([128, 2], F32)     # softplus accums per image
        dacc = sb.tile([128, 6], F32)      # dot accums per (image, cc)
        gall = sb.tile([128, 4], F32)      # TTR accums: cross0, diag0, cross1, diag1
        xrow0_t = sb.tile([1, 2304], BF16)
        xrow1_t = sb.tile([1, 2304], BF16)
        xrow_tiles = [xrow0_t, xrow1_t]
        trash = sb.tile([128, 2304], BF16)
        trashf = sb.tile([128, 1216], F32)

        # colsum psum: [128, 512]: col = b*256 + t*16 + bi (diag at bi==t)
        cs = ps_m.tile([128, 512], F32, tag="cs", bufs=1)
        nc.vector.memset(cs, 0.0)

        # ---- input DMAs ----
        # img0 first on sync; desc on gpsimd (pool idle early)
        img_tiles, sd_tiles, dsl_tiles, dnm_tiles = [], [], [], []
        img0 = sbi.tile([128, 3, 3, 384], BF16, tag="img", bufs=2)
        nc.sync.dma_start(out=img0,
                          in_=imgs_d[0].rearrange("c (hc p) w -> p c hc w", p=128))
        nc.sync.dma_start(out=b1sp_t,
                          in_=b1sp_d[:, :].rearrange("(k p) h -> p k h", p=128))
        nc.sync.dma_start(out=b1dp_t,
                          in_=b1dp_d[:, :].rearrange("(k p) h -> p k h", p=128))
        nc.sync.dma_start(out=msm_t, in_=msm_d[:, :].rearrange("(k p) h -> p k h", p=128))
        nc.sync.dma_start(out=mdf_t, in_=mdf_d[:, :].rearrange("(k p) h -> p k h", p=128))
        nc.sync.dma_start(out=mgap_t, in_=mgap_d[:, :].rearrange("(k p) h -> p k h", p=128))
        nc.sync.dma_start(out=mga_t, in_=mga_d[:, :].rearrange("(k p) h -> p k h", p=128))
        nc.sync.dma_start(out=mgan_t, in_=mgan_d[:, :].rearrange("(k p) h -> p k h", p=128))
        for b in range(NIMG):
            dnm = sbi.tile([128, 16, DDIM], FP8, tag="dnm", bufs=2)
            nc.gpsimd.dma_start(out=dnm, in_=dnm_d[b])
            dnm_tiles.append(dnm)
            dsl = sbi.tile([128, 2, NDESC], FP8, tag="dsl", bufs=2)
            nc.gpsimd.dma_start(out=dsl, in_=dsl_d[b])
            dsl_tiles.append(dsl)
        for b in range(NIMG):
            sdt = sbi.tile([128, 3, 384], BF16, tag="sdt", bufs=2)
            nc.gpsimd.dma_start(out=sdt,
                                in_=sd_d[b].rearrange("(c p) w -> p c w", p=128))
            sd_tiles.append(sdt)
        img1 = sbi.tile([128, 3, 3, 384], BF16, tag="img", bufs=2)
        nc.sync.dma_start(out=img1,
                          in_=imgs_d[1].rearrange("c (hc p) w -> p c hc w", p=128))
        img_tiles.extend([img0, img1])
        for t, d in [(s8_t, s8_d), (t16_t, t16_d), (id2_t, id2_d), (ninfh_t, ninfh_d),
                     (lw0_t, lw0_d), (e2b_t, e2b_d), (e64_t, e64_d),
                     (iota128_t, iota128_d), (thrW0_t, thrW0_d), (thrW1_t, thrW1_d),
                     (ones128_t, ones128_d), (ones1_t, ones1_d),
                     (ones64h_t, ones64h_d)]:
            nc.sync.dma_start(out=t, in_=d[:, :])

        # ---- r = rsqrt(nsq) from n-major fp8: wide square + wide reduce ----
        r_tiles = []
        dsq = sb.tile([128, 16, DDIM], BF16)

        def emit_r(b):
            dnm = dnm_tiles[b]
            nc.scalar.activation(dsq, dnm, AF.Square)
            nsqf = sbi.tile([128, 16], F32, tag="nsqf", bufs=2)
            nc.vector.tensor_reduce(nsqf, dsq, axis=AX.X, op=ALU.add)
            sr = sbi.tile([128, 16], F32, tag="sr", bufs=2)
            nc.scalar.activation(sr, nsqf, AF.Sqrt)
            y0 = sbi.tile([128, 16], F32, tag="y0", bufs=2)
            nc.vector.reciprocal(y0, sr)
            yy = sbi.tile([128, 16], F32, tag="yy", bufs=2)
            nc.vector.tensor_tensor(out=yy, in0=y0, in1=y0, op=ALU.mult)
            nc.vector.tensor_tensor(out=yy, in0=yy, in1=nsqf, op=ALU.mult)
            nc.vector.tensor_scalar(yy, yy, -0.5, 1.5, op0=ALU.mult, op1=ALU.add)
            r_bf = sbi.tile([128, 16], BF16, tag="rbf", bufs=2)
            nc.vector.tensor_tensor(out=r_bf, in0=yy, in1=y0, op=ALU.mult)
            r_tiles.append(r_bf)

        # ---- gram tile generator ----
        def gram_tiles_gen():
            for b in range(NIMG):
                for bi in range(16):
                    c0 = 128 * bi
                    pos = c0
                    while pos < NDESC:
                        wdt = min(1024, NDESC - pos)
                        yield (b, bi, pos, wdt)
                        pos += wdt

        _gram_iter = gram_tiles_gen()
        _gram_state = {"done": False, "acc": 0.0, "pending": []}

        def _emit_matvecs(ent):
            grelu, b, bi, pos, wdt = ent
            r_bf = r_tiles[b]
            for ci in range(wdt // 128):
                t = (pos + ci * 128) // 128
                col = 256 * b + 16 * t + bi
                nc.tensor.matmul(cs[:, col:col + 1],
                                 grelu[:, 128 * ci:128 * (ci + 1)],
                                 r_bf[:, bi:bi + 1],
                                 start=True, stop=True)

        def pump_gram(n, act_share=0.0):
            for _ in range(n):
                if _gram_state["done"]:
                    break
                try:
                    b, bi, pos, wdt = next(_gram_iter)
                except StopIteration:
                    _gram_state["done"] = True
                    break
                dsl = dsl_tiles[b]
                gp = ps_g.tile([128, 1024], F32, tag="g")
                off = 0
                while off < wdt:
                    nn = min(512, wdt - off)
                    nc.tensor.matmul(gp[:, off:off + nn],
                                     dsl[:, :, 128 * bi:128 * (bi + 1)],
                                     dsl[:, :, pos + off:pos + off + nn],
                                     start=True, stop=True, perf_mode=DR)
                    off += nn
                grelu = sbi.tile([128, 1024], BF16, tag="grelu", bufs=5)
                _gram_state["acc"] += act_share
                if _gram_state["acc"] >= 1.0:
                    _gram_state["acc"] -= 1.0
                    nc.scalar.activation(grelu[:, 0:wdt], gp[:, 0:wdt], AF.Relu)
                else:
                    nc.vector.tensor_scalar(grelu[:, 0:wdt], gp[:, 0:wdt], 0.0,
                                            None, op0=ALU.max)
                _gram_state["pending"].append((grelu, b, bi, pos, wdt))
                while len(_gram_state["pending"]) > 3:
                    _emit_matvecs(_gram_state["pending"].pop(0))
            if _gram_state["done"]:
                while _gram_state["pending"]:
                    _emit_matvecs(_gram_state["pending"].pop(0))

        # ---- conv + NMS per image ----
        resp_list, nms_list, bw_list = [], [], []

        def emit_conv(b, pump=True):
            img_t = img_tiles[b]
            imgv = img_t.rearrange("p c hc w -> p (c hc) w")

            # P1: smooth/diff along H -> [w-part, wb, h]
            sT = sbi.tile([128, 3, 384], BF16, tag="sT")
            dT = sbi.tile([128, 3, 384], BF16, tag="dT")
            for di, (dst, bnd, wins) in enumerate(
                    ((sT, b1sp_t, w1s), (dT, b1dp_t, w1d))):
                for wb in range(3):
                    pst = ps_cv.tile([128, 384], F32, tag="cv")
                    first = True
                    for k in range(9):
                        if wins[k] is None:
                            continue
                        c0, c1 = wins[k]
                        nc.tensor.matmul(pst[:, c0:c1],
                                         imgv[:, k, wb * 128:(wb + 1) * 128],
                                         bnd[:, k, 0:c1 - c0], start=first, stop=False)
                        first = False
                    if (di * 3 + wb) % 2 == 0:
                        nc.scalar.copy(dst[:, wb, :], pst)
                    else:
                        nc.vector.tensor_copy(dst[:, wb, :], pst)

            # P2: diff/smooth along W -> px=dx, py=dy [w-part(ob), h]
            pq = sbi.tile([128, 3, 384], BF16, tag="pq", bufs=1)
            qq = sbi.tile([128, 3, 384], BF16, tag="qq", bufs=1)
            rr = sbi.tile([128, 3, 384], BF16, tag="rr", bufs=1)
            dxs = sbi.tile([128, 384], BF16, tag="dxs", bufs=2)
            for ob in range(3):
                px = ps_cv.tile([128, 384], F32, tag="cv")
                fx = True
                for kc, ob2 in pr_df:
                    if ob2 != ob:
                        continue
                    nc.tensor.matmul(px, mdf_t[:, kc, ob * 128:(ob + 1) * 128],
                                     sT[:, kc, :], start=fx, stop=False)
                    fx = False
                nc.scalar.activation(pq[:, ob, :], px, AF.Square)
                nc.scalar.copy(dxs, px)
                py = ps_cv.tile([128, 384], F32, tag="cv")
                fy = True
                for kc, ob2 in pr_sm:
                    if ob2 != ob:
                        continue
                    nc.tensor.matmul(py, msm_t[:, kc, ob * 128:(ob + 1) * 128],
                                     dT[:, kc, :], start=fy, stop=False)
                    fy = False
                nc.scalar.activation(qq[:, ob, :], py, AF.Square)
                nc.vector.tensor_tensor(out=rr[:, ob, :], in0=dxs, in1=py, op=ALU.mult)
            if pump:
                pump_gram(2, act_share=0.2)

            # P3: gauss along W -> [h-part(hb), w]
            gP = sbi.tile([128, 3, 384], BF16, tag="gP", bufs=1)
            gM = sbi.tile([128, 3, 384], BF16, tag="gM", bufs=1)
            gR = sbi.tile([128, 3, 384], BF16, tag="gR", bufs=1)
            for si, (src, dst) in enumerate(((pq, gP), (qq, gM), (rr, gR))):
                for hb in range(3):
                    pst = ps_cv.tile([128, 384], F32, tag="cv")
                    for i, cw in enumerate(range(3)):
                        c0, c1 = wga3[cw]
                        nc.tensor.matmul(pst[:, c0:c1],
                                         src[:, cw, hb * 128:(hb + 1) * 128],
                                         mgap_t[:, cw, 0:c1 - c0],
                                         start=(i == 0), stop=False)
                    if (si * 3 + hb) % 3 == 0:
                        nc.vector.tensor_copy(dst[:, hb, :], pst)
                    else:
                        nc.scalar.copy(dst[:, hb, :], pst)

            if pump:
                pump_gram(2, act_share=0.2)

            # P4: gauss along H -> P (tr), M, R in [h-part(ob), w]; response
            resp = sbi.tile([128, 3, 388], BF16, tag="resp", bufs=2)
            for ob in range(3):
                pR = ps_cv.tile([128, 384], F32, tag="cv")
                first = True
                for kc, ob2 in pr_ga:
                    if ob2 != ob:
                        continue
                    nc.tensor.matmul(pR, mga_t[:, kc, ob * 128:(ob + 1) * 128],
                                     gR[:, kc, :], start=first, stop=False)
                    first = False
                B4 = sbi.tile([128, 384], BF16, tag="B4", bufs=2)
                nc.scalar.activation(B4, pR, AF.Square, scale=2.0)
                pM = ps_cv.tile([128, 384], F32, tag="cv")
                first = True
                for kc, ob2 in pr_ga:
                    if ob2 != ob:
                        continue
                    nc.tensor.matmul(pM, mga_t[:, kc, ob * 128:(ob + 1) * 128],
                                     gP[:, kc, :], start=first, stop=False)
                    first = False
                for kc, ob2 in pr_ga:
                    if ob2 != ob:
                        continue
                    nc.tensor.matmul(pM, mgan_t[:, kc, ob * 128:(ob + 1) * 128],
                                     gM[:, kc, :], start=False, stop=False)
                A2 = sbi.tile([128, 384], BF16, tag="A2", bufs=2)
                nc.scalar.activation(A2, pM, AF.Square)
                disc = sbi.tile([128, 384], BF16, tag="disc", bufs=2)
                nc.vector.tensor_tensor(out=disc, in0=A2, in1=B4, op=ALU.add)
                s2 = sbi.tile([128, 384], F32, tag="s2", bufs=2)
                nc.scalar.activation(s2, disc, AF.Sqrt, scale=0.25)
                pP = ps_cv.tile([128, 384], F32, tag="cv")
                first = True
                for kc, ob2 in pr_ga:
                    if ob2 != ob:
                        continue
                    nc.tensor.matmul(pP, mga_t[:, kc, ob * 128:(ob + 1) * 128],
                                     gP[:, kc, :], start=first, stop=False)
                    first = False
                for kc, ob2 in pr_ga:
                    if ob2 != ob:
                        continue
                    nc.tensor.matmul(pP, mga_t[:, kc, ob * 128:(ob + 1) * 128],
                                     gM[:, kc, :], start=False, stop=False)
                nc.vector.scalar_tensor_tensor(out=resp[:, ob, 2:386], in0=pP,
                                               scalar=0.5, in1=s2,
                                               op0=ALU.mult, op1=ALU.subtract)
                nc.vector.tensor_copy(resp[:, ob, 0:2], ninfh_t[:, 0:2])
                nc.vector.tensor_copy(resp[:, ob, 386:388], ninfh_t[:, 0:2])
            resp_list.append(resp)

        def emit_nms_gen(b):
            TTs = nc.vector.tensor_tensor
            resp = resp_list[b]
            sdt = sd_tiles[b]
            sdv = sdt.rearrange("p c w -> p (c w)")
            spA = sbi.tile([128, 1152], F32, tag="spA", bufs=1)
            nc.scalar.activation(spA, sdv, AF.Exp)
            nc.scalar.activation(trashf[:, 0:1152], spA, AF.Ln, bias=1.0,
                                 accum_out=spacc[:, b:b + 1])

            t1 = sbi.tile([128, 3, 388], BF16, tag="t1", bufs=1)
            TTs(out=t1[:, :, 0:387], in0=resp[:, :, 0:387],
                in1=resp[:, :, 1:388], op=ALU.max)
            t2 = sbi.tile([128, 3, 388], BF16, tag="t2", bufs=1)
            TTs(out=t2[:, :, 0:385], in0=t1[:, :, 0:385],
                in1=t1[:, :, 2:387], op=ALU.max)
            m1 = sbi.tile([128, 3, 384], BF16, tag="m1", bufs=2)
            TTs(out=m1, in0=t2[:, :, 0:384], in1=resp[:, :, 4:388], op=ALU.max)
            yield
            shs = []
            for k in (1, 2):
                sh = sbi.tile([128, 3, 384], BF16, tag="shd", bufs=2)
                nc.vector.memset(sh[0:k, 0, :], NEG)
                nc.sync.dma_start(out=sh[k:128], in_=m1[0:128 - k])
                nc.sync.dma_start(out=sh[0:k, 1:3, :], in_=m1[128 - k:128, 0:2, :])
                shs.append(sh)
                sh2 = sbi.tile([128, 3, 384], BF16, tag="shu", bufs=2)
                nc.gpsimd.dma_start(out=sh2[128 - k:128, 2, :],
                                    in_=ninfh_t[0:k, 0:384])
                nc.gpsimd.dma_start(out=sh2[0:128 - k], in_=m1[k:128])
                nc.gpsimd.dma_start(out=sh2[128 - k:128, 0:2, :], in_=m1[0:k, 1:3, :])
                shs.append(sh2)
            yield
            mp1 = sbi.tile([128, 3, 384], BF16, tag="mp1", bufs=1)
            nc.vector.tensor_tensor(out=mp1, in0=m1, in1=shs[0], op=ALU.max)
            mp2 = sbi.tile([128, 3, 384], BF16, tag="mp2", bufs=1)
            TTs(out=mp2, in0=shs[1], in1=shs[2], op=ALU.max)
            nc.vector.tensor_tensor(out=mp2, in0=mp2, in1=shs[3], op=ALU.max)
            mp = sbi.tile([128, 3, 384], BF16, tag="mp", bufs=2)
            nc.vector.tensor_tensor(out=mp, in0=mp1, in1=mp2, op=ALU.max)
            yield
            e1 = sbi.tile([128, 3, 384], BF16, tag="e1", bufs=1)
            TTs(out=e1, in0=resp[:, :, 2:386], in1=mp, op=ALU.is_ge)
            nms = sbi.tile([128, 3, 384], BF16, tag="nms", bufs=2)
            nc.vector.tensor_tensor(out=nms, in0=resp[:, :, 2:386], in1=e1, op=ALU.mult)
            nms_list.append(nms)
            yield
            bw = sbi.tile([128, 3, 48], BF16, tag="bw")
            nc.vector.tensor_reduce(bw, nms.rearrange("p c (g k) -> p c g k", k=8),
                                    axis=AX.X, op=ALU.max)
            shbs = []
            for k in range(1, 8):
                shk = sbi.tile([128, 3, 48], BF16, tag="shb", bufs=8)
                q = nc.sync if k % 2 else nc.gpsimd
                q.dma_start(out=shk[0:121], in_=bw[k:121 + k])
                shbs.append(shk)
            yield
            q1 = sbi.tile([128, 3, 48], BF16, tag="bwm", bufs=2)
            nc.vector.tensor_tensor(out=q1[0:121], in0=bw[0:121], in1=shbs[0][0:121],
                                    op=ALU.max)
            q2 = sbi.tile([128, 3, 48], BF16, tag="bwm", bufs=2)
            nc.vector.tensor_tensor(out=q2[0:121], in0=shbs[1][0:121],
                                    in1=shbs[2][0:121], op=ALU.max)
            q3 = sbi.tile([128, 3, 48], BF16, tag="bwm3", bufs=2)
            nc.vector.tensor_tensor(out=q3[0:121], in0=shbs[3][0:121],
                                    in1=shbs[4][0:121], op=ALU.max)
            q4 = sbi.tile([128, 3, 48], BF16, tag="bwm3", bufs=2)
            nc.vector.tensor_tensor(out=q4[0:121], in0=shbs[5][0:121],
                                    in1=shbs[6][0:121], op=ALU.max)
            q5 = sbi.tile([128, 3, 48], BF16, tag="bwm5", bufs=2)
            nc.vector.tensor_tensor(out=q5[0:121], in0=q1[0:121], in1=q2[0:121],
                                    op=ALU.max)
            q6 = sbi.tile([128, 3, 48], BF16, tag="bwm5", bufs=2)
            nc.vector.tensor_tensor(out=q6[0:121], in0=q3[0:121], in1=q4[0:121],
                                    op=ALU.max)
            yield
            cur = sbi.tile([128, 3, 48], BF16, tag="bwf", bufs=2)
            nc.vector.tensor_tensor(out=cur[0:121], in0=q5[0:121], in1=q6[0:121],
                                    op=ALU.max)
            yield
            p16 = ps_m.tile([16, 144], F32, tag="m", bufs=1)
            nc.tensor.matmul(p16, s8_t[0:121, :],
                             cur[0:121].rearrange("p c g -> p (c g)"),
                             start=True, stop=True)
            p16s = sbi.tile([16, 3, 48], BF16, tag="p16s", bufs=2)
            nc.scalar.copy(p16s.rearrange("p c g -> p (c g)"), p16)
            bw_list.append(p16s)
            xr = xrow_tiles[b]
            p16f = p16s.rearrange("p c g -> p (c g)")
            nc.sync.dma_start(out=xr[0:1, 0:1152], in_=p16f[0:8, :])
            nc.gpsimd.dma_start(out=xr[0:1, 1152:2304], in_=p16f[8:16, :])

        # ---- per-image threshold search + selection (interleavable) ----
        W0 = 0.25 / 64.0
        W1 = 0.25 / 4096.0
        maskz = sb.tile([128, 2, 2], F32)   # [*, img, round] zero-padded masks
        nc.vector.memset(maskz, 0.0)
        trash64 = sb.tile([64, 2304], BF16)

        def thresh_sel_gen_img(b):
            xr = xrow_tiles[b]
            x64 = sbi.tile([64, 2304], BF16, tag="x64", bufs=2)
            nc.gpsimd.partition_broadcast(x64, xr, channels=64)
            yield
            # round 1 (constant thresholds)
            cnt = sbi.tile([64, 1], F32, tag="cnt", bufs=2)
            nc.vector.tensor_scalar(trash64, x64, thrW0_t[:, 0:1], None,
                                    op0=ALU.is_gt, op1=ALU.add, accum_out=cnt)
            nc.vector.tensor_scalar(maskz[0:64, b, 0:1], cnt, float(NUM) - 0.5,
                                    None, op0=ALU.is_ge)
            yield
            kp1 = ps_m.tile([1, 1], F32, tag="m", bufs=1)
            nc.tensor.matmul(kp1, maskz[:, b, 0:1], ones128_t, start=True, stop=True)
            lo_sc = sbi.tile([1, 1], F32, tag="losc", bufs=2)
            nc.vector.tensor_scalar(lo_sc, kp1, W0, -W0, op0=ALU.mult, op1=ALU.add)
            lo64p = ps_m.tile([64, 1], F32, tag="m", bufs=1)
            nc.tensor.matmul(lo64p, ones1[0:1, 0:64] if False else ones1_t[0:1, 0:64],
                             lo_sc, start=True, stop=True)
            T2 = sbi.tile([64, 1], F32, tag="T2", bufs=2)
            nc.vector.tensor_tensor(out=T2, in0=lo64p, in1=thrW1_t, op=ALU.add)
            yield
            # round 2
            cnt2 = sbi.tile([64, 1], F32, tag="cnt2", bufs=2)
            nc.vector.tensor_scalar(trash64, x64, T2[:, 0:1], None,
                                    op0=ALU.is_gt, op1=ALU.add, accum_out=cnt2)
            nc.vector.tensor_scalar(maskz[0:64, b, 1:2], cnt2, float(NUM) - 0.5,
                                    None, op0=ALU.is_ge)
            yield
            kp2 = ps_m.tile([1, 1], F32, tag="m", bufs=1)
            nc.tensor.matmul(kp2, maskz[:, b, 1:2], ones128_t, start=True, stop=True)
            tf = sbi.tile([1, 1], F32, tag="tf", bufs=2)
            nc.vector.tensor_scalar(tf, kp2, W1, -W1, op0=ALU.mult, op1=ALU.add)
            nc.vector.tensor_tensor(out=tf, in0=tf, in1=lo_sc, op=ALU.add)
            nc.vector.tensor_scalar(tf, tf, 1e-30, None, op0=ALU.max)
            tb16p = ps_m.tile([16, 1], F32, tag="m", bufs=1)
            nc.tensor.matmul(tb16p, ones1_t[0:1, 0:16], tf, start=True, stop=True)
            tb16 = sbi.tile([16, 1], F32, tag="tb16", bufs=2)
            nc.scalar.copy(tb16, tb16p)
            yield
            # selection + dot
            p16s = bw_list[b]
            nms = nms_list[b]
            sdt = sd_tiles[b]
            p16c = sbi.tile([16, 3, 48], BF16, tag="p16c", bufs=2)
            nc.vector.tensor_scalar(p16c.rearrange("p c g -> p (c g)"),
                                    p16s.rearrange("p c g -> p (c g)"),
                                    tb16[:, 0:1], None, op0=ALU.max)
            yield
            for cc in range(3):
                bexp = ps_m.tile([128, 384], F32, tag="m", bufs=1)
                nc.tensor.matmul(bexp, t16_t,
                                 p16c[:, cc, :].unsqueeze(2)
                                 .to_broadcast([16, 48, 8]),
                                 start=True, stop=True)
                sel = sbi.tile([128, 384], BF16, tag="sel", bufs=2)
                nc.vector.tensor_tensor(out=sel, in0=nms[:, cc, :], in1=bexp,
                                        op=ALU.is_ge)
                dtmp = sbi.tile([128, 384], BF16, tag="dtmp", bufs=2)
                nc.vector.tensor_tensor(out=dtmp, in0=sel, in1=sdt[:, cc, :],
                                        op=ALU.mult)
                nc.vector.tensor_reduce(dacc[:, 3 * b + cc:3 * b + cc + 1],
                                        dtmp, axis=AX.X, op=ALU.add)
                yield

        # ================= schedule =================
        def drive(gen, tiles_per_step=1, act_share=0.45):
            for _ in gen:
                pump_gram(tiles_per_step, act_share=act_share)

        emit_conv(0, pump=False)
        pump_gram(3)
        emit_r(0)
        emit_r(1)
        drive(emit_nms_gen(0), 1, 0.3)
        emit_conv(1)
        drive(thresh_sel_gen_img(0), 0, 0.4)
        drive(emit_nms_gen(1), 2, 0.7)
        drive(thresh_sel_gen_img(1), 2, 0.7)
        pump_gram(1000, act_share=0.7)

        # gram weighted sums per image: full (incl diag) and diag-only
        for b in range(NIMG):
            wcs = sbi.tile([128, 256], F32, tag="wcs", bufs=2)
            nc.vector.tensor_tensor(
                out=wcs.rearrange("p (a k) -> p a k", k=16),
                in0=cs[:, 256 * b:256 * b + 256].rearrange("p (a k) -> p a k", k=16),
                in1=r_tiles[b].unsqueeze(2).to_broadcast([128, 16, 16]),
                op=ALU.mult)
            nc.vector.tensor_reduce(gall[:, 2 * b:2 * b + 1], wcs,
                                    axis=AX.X, op=ALU.add)
            wcd = sbi.tile([128, 16], F32, tag="wcd", bufs=2)
            import concourse.ap as ap_mod
            csap = cs[:, 256 * b:256 * b + 256]
            diag_ap = ap_mod.AP(csap.tensor, csap.offset,
                                [list(csap.ap[0]), [17 * csap.ap[1][0], 16]])
            nc.vector.tensor_tensor(out=wcd, in0=diag_ap, in1=r_tiles[b],
                                    op=ALU.mult)
            nc.vector.tensor_reduce(gall[:, 2 * b + 1:2 * b + 2], wcd,
                                    axis=AX.X, op=ALU.add)

        # ---- final reduce ----
        vals = sb.tile([128, 4], F32)
        nc.vector.tensor_reduce(vals[:, 0:1], spacc, axis=AX.X, op=ALU.add)
        nc.vector.tensor_reduce(vals[:, 1:2], dacc, axis=AX.X, op=ALU.add)
        nc.vector.tensor_tensor(out=vals[:, 2:3], in0=gall[:, 0:1], in1=gall[:, 2:3],
                                op=ALU.add)
        nc.vector.tensor_tensor(out=vals[:, 3:4], in0=gall[:, 1:2], in1=gall[:, 3:4],
                                op=ALU.add)
        fps = ps_m.tile([4, 1], F32, tag="m", bufs=1)
        nc.tensor.matmul(fps, vals, ones128_t, start=True, stop=True)
        fsb = sb.tile([4, 1], F32)
        nc.scalar.copy(fsb, fps)
        nc.sync.dma_start(out=out_d[:, :], in_=fsb)

        ps_m.release()
        ps_g.release()
        ps_cv.release()
        sbi.release()
        sb.release()

    nc.finalize()
    return nc, C


_CACHE = {}


def kernel(descriptors, scores, scores_dense, imgs):
    B = descriptors.shape[0]
    ncore = 8
    per = B // ncore
    if "nc" not in _CACHE:
        _CACHE["nc"], _CACHE["C"] = build_program()
    nc, C = _CACHE["nc"], _CACHE["C"]

    imgs_bf = np.ascontiguousarray(np.asarray(imgs).astype(ml_dtypes.bfloat16))
    sd = np.ascontiguousarray(np.asarray(scores_dense).reshape(B, H, W)
                              .astype(ml_dtypes.bfloat16))
    desc8 = np.asarray(descriptors).astype(ml_dtypes.float8_e4m3)
    # slab d-major: [B, 128(dj), 2(slab), N]
    dsl = np.ascontiguousarray(desc8.transpose(0, 2, 1)
                               .reshape(B, 2, 128, NDESC).transpose(0, 2, 1, 3))
    # n-major: [B, 128(n in chunk), 16(chunk), D]
    dnm = np.ascontiguousarray(desc8.reshape(B, 16, 128, DDIM).transpose(0, 2, 1, 3))

    in_maps = []
    for c in range(ncore):
        s = slice(c * per, (c + 1) * per)
        in_maps.append({
            "imgs": imgs_bf[s], "sd": sd[s], "dsl": dsl[s], "dnm": dnm[s],
            "b1sp": C["b1sp"], "b1dp": C["b1dp"], "mgap": C["mgap"],
            "msm": C["msm"], "mdf": C["mdf"], "mga": C["mga"], "mgan": C["mgan"],
            "s8": C["s8"], "t16": C["t16"], "id2": C["id2"], "ninfh": C["ninfh"],
            "lw0": C["lw0"], "e2b": C["e2b"], "e64": C["e64"],
            "iota128": C["iota128"], "thrW0": C["thrW0"], "thrW1": C["thrW1"],
            "ones128": C["ones128"], "ones1": C["ones1"],
            "ones64h": C["ones64h"],
        })

    res = run_bass_kernel_spmd(nc, in_maps, core_ids=list(range(ncore)))
    S1 = S2 = Sall = Sdia = 0.0
    for c in range(ncore):
        o = np.asarray(res.results[c]["out"])[:, 0].astype(np.float64)
        S1 += o[0]
        S2 += o[1]
        Sall += o[2]
        Sdia += o[3]
    bce = (S1 - S2) / (B * H * W)
    relu_mean = (2.0 * Sall - Sdia) / (B * NDESC * NDESC)
    return np.array(bce + relu_mean, dtype=np.float32)
